# revision 1
# baseline (speedup 1.0000x reference)
"""Trainium2 Bass kernel for FFNWithScales (SwiGLU MLP with low-rank dequant scales).

Reference computation (all fp32):
    gate_eff = gate_snapped * (gate_scale_A @ gate_scale_B)       # [8192, 2048]
    up_eff   = up_snapped   * (up_scale_A   @ up_scale_B)         # [8192, 2048]
    down_eff = down_snapped * (down_scale_A @ down_scale_B)       # [2048, 8192]
    h   = silu(gate_eff @ x) * (up_eff @ x)                       # [8192, 512]
    out = down_eff @ h                                            # [2048, 512]

Sharding (8 cores, tensor-parallel on d_ff): core c owns d_ff rows
[c*1024, (c+1)*1024) of gate/up (and the matching columns of down).
Each core computes a full-[2048, 512] partial of the down projection;
partials are summed on the host (the all-reduce step).

Device notes:
  - PE matmul computes psum[M,N] = lhsT[K,M].T @ rhs[K,N] with K on
    partitions, so every weight is fed with its contraction dim on
    partitions. The host pre-transposes the snapped weights (one numpy
    transpose each) because fp32 has no DMA-transpose path on TRN2.
  - The fp32 snapped weights (24 MiB/core — the dominant HBM traffic)
    stream through in [128, 2, 512] pairs: one 512 KiB DMA, a packed
    pair of rank-32 scale matmuls (row-tiled via tile_position so both
    run concurrently in the PE array), one DVE dequant multiply that
    rounds to bf16, then eight [128,128]x[128,512] bf16 main matmuls
    with fp32 psum accumulation. bf16 streams ~3x faster than fp32r on
    the PE, which is what makes the kernel DMA-bound. Measured
    end-to-end error vs the fp32 reference: ~5e-3 of output absmax.
  - The broadcast activations x and the rank-32 factors are shipped
    bf16 in their final device layouts (host prep), so no on-device
    staging/rounding chain exists to stall the weight pipeline.
  - DMA rings: sync HWDGE carries only the weight stream (HWDGE is
    FIFO per issuing engine — a waiting DMA would head-of-line block
    the stream), scalar HWDGE carries the small constant loads, and
    output stores go out the gpsimd SWDGE ring.
  - Each pass's first scale-pack/dequant is emitted before the
    previous pass's epilogue so pass boundaries only wait on psum
    accumulator release.
"""

import numpy as np
import ml_dtypes

import concourse.bass as bass
from concourse import bacc
import concourse.mybir as mybir
from concourse.tile import TileContext
from concourse.bass_utils import run_bass_kernel_spmd

P = 128
D = 2048        # d_model
FF = 8192       # d_ff (global)
S = 512         # sequence
R = 32          # rank
NCORES = 8
F = FF // NCORES          # 1024 local d_ff rows
KD = D // P               # 16 d_model chunks
KF = F // P               # 8 local d_ff chunks
FG = 512                  # free-dim group (psum bank width)

f32 = mybir.dt.float32
bf16 = mybir.dt.bfloat16

_CACHE = {}


def _build():
    nc = bacc.Bacc()
    # x / scale factors arrive bf16 in device layout; weights arrive fp32.
    x = nc.declare_dram_parameter("x", [D, S], bf16, isOutput=False)
    gT = nc.declare_dram_parameter("gT", [D, F], f32, isOutput=False)
    uT = nc.declare_dram_parameter("uT", [D, F], f32, isOutput=False)
    dT = nc.declare_dram_parameter("dT", [F, D], f32, isOutput=False)
    # B2 [64, nk/2, 128]: strip i holds B cols for kd-chunk 2*kp+i (lhsT of
    # the packed scale matmul); AT2 [64, w]: A^T replicated on both strips.
    gB2 = nc.declare_dram_parameter("gB2", [2 * R, KD // 2, P], bf16, isOutput=False)
    uB2 = nc.declare_dram_parameter("uB2", [2 * R, KD // 2, P], bf16, isOutput=False)
    dB2 = nc.declare_dram_parameter("dB2", [2 * R, KF // 2, P], bf16, isOutput=False)
    gAT2 = nc.declare_dram_parameter("gAT2", [2 * R, F], bf16, isOutput=False)
    uAT2 = nc.declare_dram_parameter("uAT2", [2 * R, F], bf16, isOutput=False)
    dAT2 = nc.declare_dram_parameter("dAT2", [2 * R, D], bf16, isOutput=False)
    out = nc.declare_dram_parameter("out", [D, S], f32, isOutput=True)

    with TileContext(nc) as tc:
        with (
            tc.tile_pool(name="const", bufs=1) as const,
            tc.tile_pool(name="wstream", bufs=14) as wpool,
            tc.tile_pool(name="hbuf", bufs=1) as hpool,
            tc.tile_pool(name="obuf", bufs=3) as opool,
            tc.tile_pool(name="psacc", bufs=1, space="PSUM") as psacc,
            tc.tile_pool(name="pssc", bufs=2, space="PSUM") as pssc,
        ):
            # Startup critical path: the first scale-pack needs the gate
            # factors and the first mains need x chunk 0, so those lead the
            # sync ring right before the weight stream; everything else
            # loads on the scalar ring.
            rounded = {}

            def load_factor(nm, dram, eng):
                rt = const.tile(list(dram.shape), bf16, name=f"{nm}r", tag=f"{nm}r")
                eng.dma_start(rt, dram[:])
                rounded[nm] = rt

            load_factor("gB", gB2, nc.sync)
            load_factor("gAT", gAT2, nc.scalar)

            XC = 2
            x_sb = [None] * (KD // XC)

            def load_x_chunk(q, eng):
                xt = const.tile([P, XC, S], bf16, name=f"x{q}", tag=f"x{q}")
                eng.dma_start(
                    xt, x[q * XC * P:(q + 1) * XC * P, :].rearrange(
                        "(ko p) s -> p ko s", p=P))
                x_sb[q] = xt

            def xs(kd):
                return x_sb[kd // XC][:, kd % XC]

            load_x_chunk(0, nc.sync)
            load_x_chunk(1, nc.scalar)

            load_factor("uB", uB2, nc.gpsimd)
            load_factor("uAT", uAT2, nc.gpsimd)
            load_factor("dBs", dB2, nc.gpsimd)
            load_factor("dAT", dAT2, nc.gpsimd)

            # h = silu(gate) * up, [128, 8, 512] resident
            h_sb = hpool.tile([P, KF, S], bf16)

            silu = mybir.ActivationFunctionType.Silu

            def gate_up_finish(acc, fg, is_up):
                for fi in range(4):
                    f = fg * 4 + fi
                    if is_up:
                        nc.vector.tensor_mul(
                            out=h_sb[:, f], in0=h_sb[:, f], in1=acc[fi])
                    else:
                        nc.scalar.activation(h_sb[:, f], acc[fi], silu)

            def down_finish(acc, mg):
                if mg < D // FG - 1:
                    # two batched [128, 2, 512] stores; the copy runs right
                    # before its store at the same program position, so the
                    # HWDGE store can't head-of-line block the remaining
                    # weight stream for long.
                    for half in range(2):
                        ot2 = opool.tile([P, 2, S], f32, name="ot", tag="ot")
                        for j in range(2):
                            nc.scalar.copy(ot2[:, j], acc[half * 2 + j])
                        weng = nc.sync if half == 0 else nc.scalar
                        weng.dma_start(
                            out[(mg * 4 + half * 2) * P:
                                (mg * 4 + half * 2 + 2) * P, :].rearrange(
                                "(mo p) s -> p mo s", p=P), ot2)
                else:
                    # final pass: this epilogue is the kernel tail, so drain
                    # it wide — copies split across ACT and DVE (both idle by
                    # now), four small stores across both HWDGE rings.
                    for mi in range(4):
                        ot = opool.tile([P, S], f32, name="otl", tag="otl")
                        if mi % 2 == 0:
                            nc.scalar.copy(ot, acc[mi])
                        else:
                            nc.vector.tensor_copy(out=ot, in_=acc[mi])
                        weng = nc.sync if mi % 2 == 0 else nc.scalar
                        weng.dma_start(
                            out[(mg * 4 + mi) * P:(mg * 4 + mi + 1) * P, :],
                            ot)

            passes = []
            for is_up in (0, 1):
                for fg in range(F // FG):
                    passes.append(dict(
                        wdram=uT if is_up else gT,
                        Bn="uB" if is_up else "gB",
                        An="uAT" if is_up else "gAT",
                        nk=KD, fg=fg, rhs_fn=xs,
                        finish=lambda acc, fg=fg, is_up=is_up:
                            gate_up_finish(acc, fg, is_up),
                    ))
            for mg in range(D // FG):
                passes.append(dict(
                    wdram=dT, Bn="dBs", An="dAT",
                    nk=KF, fg=mg, rhs_fn=lambda kf: h_sb[:, kf],
                    finish=lambda acc, mg=mg: down_finish(acc, mg),
                ))

            sc_tiles = {}

            def emit_sc(pi, kp):
                ps = passes[pi]
                fg = ps["fg"]
                sc2 = pssc.tile([P, 2, FG], f32, name="sc", tag="sc")
                for i in range(2):
                    nc.tensor.matmul(
                        sc2[:, i],
                        rounded[ps["Bn"]][i * R:(i + 1) * R, kp],
                        rounded[ps["An"]][i * R:(i + 1) * R,
                                          fg * FG:(fg + 1) * FG],
                        start=True, stop=True,
                        tile_position=(R * i, 0),
                    )
                sc_tiles[pi, kp] = sc2

            wr_tiles = {}

            wt_tiles = {}

            def emit_wt(pi, kp):
                """Weight DMA for pair (pi, kp). Issued several jobs ahead
                of the dequant (no psum involved) so a slow transfer can't
                starve the PE."""
                ps = passes[pi]
                fg = ps["fg"]
                wt2 = wpool.tile([P, 2, FG], f32, name="wt", tag="wt")
                # alternate the weight stream across both HWDGE rings so
                # back-to-back pairs overlap their transfer+completion
                # latency instead of serializing on one ring.
                weng = nc.sync if (pi + kp) % 2 == 0 else nc.scalar
                weng.dma_start(
                    wt2,
                    ps["wdram"][kp * 2 * P:(kp + 1) * 2 * P,
                                fg * FG:(fg + 1) * FG].rearrange(
                                    "(ko p) f -> p ko f", p=P))
                wt_tiles[pi, kp] = wt2

            def emit_dequant(pi, kp):
                wr2 = wpool.tile([P, 2, FG], bf16, name="wr", tag="wr")
                nc.vector.tensor_mul(out=wr2, in0=wt_tiles.pop((pi, kp)),
                                     in1=sc_tiles.pop((pi, kp)))
                wr_tiles[pi, kp] = wr2

            # Flat pair-job list across all passes, software-pipelined with
            # TWO pairs of scale-pack/dequant look-ahead (2 sc psum slots
            # cover it: job J+2's pack allocates the slot job J's dequant
            # just freed). The deeper pipeline absorbs DVE timing jitter at
            # pair and pass boundaries.
            jobs = []
            for pi, ps in enumerate(passes):
                for kp in range(ps["nk"] // 2):
                    jobs.append((pi, kp))
            LOOKAHEAD = 2
            DMA_AHEAD = 6
            for J in range(DMA_AHEAD):
                emit_wt(*jobs[J])
            for J in range(LOOKAHEAD):
                emit_sc(*jobs[J])
                emit_dequant(*jobs[J])

            cur_acc = {}
            for J, (pi, kp) in enumerate(jobs):
                ps = passes[pi]
                npairs = ps["nk"] // 2
                if kp == 0:
                    cur_acc[pi] = [
                        psacc.tile([P, S], f32, name=f"acc{i}", tag=f"acc{i}")
                        for i in range(4)]
                if pi == 0 and 2 <= kp + 2 < KD // XC:
                    # pull the rest of x in just-in-time on the ring the
                    # weight stream isn't using this iteration (chunk q
                    # is first consumed at pair kp=q).
                    load_x_chunk(
                        kp + 2,
                        nc.scalar if (pi + kp) % 2 == 0 else nc.sync)
                if J + DMA_AHEAD < len(jobs):
                    emit_wt(*jobs[J + DMA_AHEAD])
                if J + LOOKAHEAD < len(jobs):
                    emit_sc(*jobs[J + LOOKAHEAD])
                    emit_dequant(*jobs[J + LOOKAHEAD])
                acc = cur_acc[pi]
                wr2 = wr_tiles.pop((pi, kp))
                for j in range(2):
                    for fi in range(4):
                        nc.tensor.matmul(
                            acc[fi],
                            wr2[:, j, fi * P:(fi + 1) * P],
                            ps["rhs_fn"](2 * kp + j),
                            start=(kp == 0 and j == 0),
                            stop=(kp == npairs - 1 and j == 1),
                        )
                if kp == npairs - 1:
                    ps["finish"](cur_acc.pop(pi))
    nc.finalize()
    return nc


def _prep_inputs(x, gate_snapped, gate_scale_A, gate_scale_B,
                 up_snapped, up_scale_A, up_scale_B,
                 down_snapped, down_scale_A, down_scale_B):
    asf = lambda a: np.ascontiguousarray(np.asarray(a, dtype=np.float32))
    bf = ml_dtypes.bfloat16
    x2 = np.ascontiguousarray(np.asarray(x, dtype=np.float32).reshape(D, S)
                              .astype(bf))
    gT_full = asf(gate_snapped).T      # [D, FF] view
    uT_full = asf(up_snapped).T
    dT_full = asf(down_snapped).T      # [FF, D] view

    def pack_B2(Bmat, nk):
        # [R, nk*128] -> [64, nk/2, 128]: strip i holds chunks 2*kp+i
        b = np.asarray(Bmat, dtype=np.float32).reshape(R, nk // 2, 2, P)
        o = np.empty((2 * R, nk // 2, P), dtype=bf)
        o[:R] = b[:, :, 0, :].astype(bf)
        o[R:] = b[:, :, 1, :].astype(bf)
        return o

    def pack_AT2(Amat):
        # A [w, R] -> A^T [R, w] replicated on both strips -> [64, w]
        at = np.asarray(Amat, dtype=np.float32).T.astype(bf)
        return np.ascontiguousarray(np.concatenate([at, at], axis=0))

    gB_f = np.asarray(gate_scale_B, dtype=np.float32)
    uB_f = np.asarray(up_scale_B, dtype=np.float32)
    dB_f = np.asarray(down_scale_B, dtype=np.float32)
    gA_f = np.asarray(gate_scale_A, dtype=np.float32)
    uA_f = np.asarray(up_scale_A, dtype=np.float32)
    dAT2 = pack_AT2(down_scale_A)      # [64, D]

    in_maps = []
    for c in range(NCORES):
        lo, hi = c * F, (c + 1) * F
        in_maps.append({
            "x": x2,
            "gT": np.ascontiguousarray(gT_full[:, lo:hi]),
            "uT": np.ascontiguousarray(uT_full[:, lo:hi]),
            "dT": np.ascontiguousarray(dT_full[lo:hi, :]),
            "gB2": pack_B2(gB_f, KD),
            "uB2": pack_B2(uB_f, KD),
            "dB2": pack_B2(dB_f[:, lo:hi], KF),
            "gAT2": pack_AT2(gA_f[lo:hi]),
            "uAT2": pack_AT2(uA_f[lo:hi]),
            "dAT2": dAT2,
        })
    return in_maps


def run(trace=False, **inputs):
    if "nc" not in _CACHE:
        _CACHE["nc"] = _build()
    nc = _CACHE["nc"]
    in_maps = _prep_inputs(**inputs)
    try:
        res = run_bass_kernel_spmd(nc, in_maps, list(range(NCORES)), trace=trace)
    except Exception:
        # A transient device flake (NRT_EXEC_UNIT_UNRECOVERABLE) poisons the
        # PJRT client for the process; tearing the backend down and
        # reconnecting recovers it the same way a fresh process does.
        try:
            import jax.extend.backend
            jax.extend.backend.clear_backends()
        except Exception:
            pass
        res = run_bass_kernel_spmd(nc, in_maps, list(range(NCORES)), trace=trace)
    partial = np.zeros((D, S), dtype=np.float32)
    for c in range(NCORES):
        partial += res.results[c]["out"]
    return partial.reshape(1, D, 1, S), res


def kernel(**inputs):
    out, _ = run(trace=False, **inputs)
    return out


if __name__ == "__main__":
    rng = np.random.default_rng(0)
    ins = {
        "x": rng.standard_normal((1, D, 1, S)).astype(np.float32),
        "gate_snapped": (rng.standard_normal((FF, D)) * 0.02).astype(np.float32),
        "gate_scale_A": (rng.standard_normal((FF, R)) * 0.1).astype(np.float32),
        "gate_scale_B": (rng.standard_normal((R, D)) * 0.1).astype(np.float32),
        "up_snapped": (rng.standard_normal((FF, D)) * 0.02).astype(np.float32),
        "up_scale_A": (rng.standard_normal((FF, R)) * 0.1).astype(np.float32),
        "up_scale_B": (rng.standard_normal((R, D)) * 0.1).astype(np.float32),
        "down_snapped": (rng.standard_normal((D, FF)) * 0.02).astype(np.float32),
        "down_scale_A": (rng.standard_normal((D, R)) * 0.1).astype(np.float32),
        "down_scale_B": (rng.standard_normal((R, FF)) * 0.1).astype(np.float32),
    }
    out = kernel(**ins)
    print("kernel ran, out shape", out.shape, "mean abs", np.abs(out).mean())



# revision 2
# speedup vs baseline: 1.1024x; 1.1024x over previous
"""Trainium2 Bass kernel for FFNWithScales (SwiGLU MLP with low-rank dequant scales).

Reference computation (all fp32):
    gate_eff = gate_snapped * (gate_scale_A @ gate_scale_B)       # [8192, 2048]
    up_eff   = up_snapped   * (up_scale_A   @ up_scale_B)         # [8192, 2048]
    down_eff = down_snapped * (down_scale_A @ down_scale_B)       # [2048, 8192]
    h   = silu(gate_eff @ x) * (up_eff @ x)                       # [8192, 512]
    out = down_eff @ h                                            # [2048, 512]

Sharding (8 cores, tensor-parallel on d_ff): core c owns d_ff rows
[c*1024, (c+1)*1024) of gate/up (and the matching columns of down).
Each core computes a full-[2048, 512] partial of the down projection;
bf16 partials are summed in fp32 on the host (the all-reduce step).

Device notes:
  - PE matmul computes psum[M,N] = lhsT[K,M].T @ rhs[K,N] with K on
    partitions. The host ships everything bf16 in final device layout:
    snapped weights are pre-transposed AND pre-tiled so each weight DMA
    is one fully contiguous [128, 4, 512] (512 KiB) "quad" = 4 K-chunks
    of one 512-wide output group. bf16 weights halve the dominant HBM
    stream vs fp32 (measured end-to-end error ~7e-3 of output absmax,
    threshold 2e-2).
  - Work unit = quad job: one 512 KiB weight DMA, one 4-way row-packed
    rank-32 scale matmul (strips at tile_position (32i,0) run
    concurrently in the PE array, output [128,4,512] psum = 4 banks),
    one full-tile DVE dequant multiply (bf16), then 16 [128,128]x
    [128,512] bf16 main matmuls accumulating into 4 psum banks.
    PSUM budget: 4 acc banks + 4 scale banks = all 8.
  - DMA rings: sync+scalar HWDGE alternate the weight stream (and carry
    x / output stores interleaved); the six small scale-factor loads go
    on the gpsimd SWDGE ring so the startup critical path is just
    factors(gpsimd) -> scale mm -> dequant in parallel with wt+x DMAs.
  - Up-projection finish copies psum to bf16 SBUF on ACT, then the
    SwiGLU multiply runs SBUF-only on DVE in 2x packed mode, keeping
    DVE's steady-state load (the dequants) off the critical path at
    pass boundaries.
"""

import numpy as np
import ml_dtypes

import concourse.bass as bass
from concourse import bacc
import concourse.mybir as mybir
from concourse.tile import TileContext
from concourse.bass_utils import run_bass_kernel_spmd

P = 128
D = 2048        # d_model
FF = 8192       # d_ff (global)
S = 512         # sequence
R = 32          # rank
NCORES = 8
F = FF // NCORES          # 1024 local d_ff rows
KD = D // P               # 16 d_model chunks
KF = F // P               # 8 local d_ff chunks
FG = 512                  # free-dim group (psum bank width)

f32 = mybir.dt.float32
bf16 = mybir.dt.bfloat16

_CACHE = {}


def _build():
    nc = bacc.Bacc()
    x = nc.declare_dram_parameter("x", [P, KD, S], bf16, isOutput=False)
    # quad-tiled weights: [pass*quads, 128, 4 k-chunks, 512 out-cols]
    gTp = nc.declare_dram_parameter("gTp", [8, P, 4, FG], bf16, isOutput=False)
    uTp = nc.declare_dram_parameter("uTp", [8, P, 4, FG], bf16, isOutput=False)
    dTp = nc.declare_dram_parameter("dTp", [8, P, 4, FG], bf16, isOutput=False)
    # B4 [128, nk/4, 128]: strip i (partitions 32i..32i+31) holds scale-B
    # cols for chunk 4q+i; AT4 [128, w]: A^T replicated on all 4 strips.
    gB4 = nc.declare_dram_parameter("gB4", [P, KD // 4, P], bf16, isOutput=False)
    uB4 = nc.declare_dram_parameter("uB4", [P, KD // 4, P], bf16, isOutput=False)
    dB4 = nc.declare_dram_parameter("dB4", [P, KF // 4, P], bf16, isOutput=False)
    gAT4 = nc.declare_dram_parameter("gAT4", [P, F], bf16, isOutput=False)
    uAT4 = nc.declare_dram_parameter("uAT4", [P, F], bf16, isOutput=False)
    dAT4 = nc.declare_dram_parameter("dAT4", [P, D], bf16, isOutput=False)
    out = nc.declare_dram_parameter("out", [4, P, 4, S], bf16, isOutput=True)

    silu = mybir.ActivationFunctionType.Silu

    with TileContext(nc) as tc:
        with (
            tc.tile_pool(name="const", bufs=1) as const,
            tc.tile_pool(name="wstream", bufs=5) as wpool,
            tc.tile_pool(name="wready", bufs=3) as rpool,
            tc.tile_pool(name="utmp", bufs=2) as upool,
            tc.tile_pool(name="obuf", bufs=2) as opool,
            tc.tile_pool(name="psacc", bufs=1, space="PSUM") as psacc,
            tc.tile_pool(name="pssc", bufs=1, space="PSUM") as pssc,
        ):
            # All six factor loads ride the otherwise-idle SWDGE ring;
            # gate factors first (they gate the first scale matmul).
            fac = {}
            for nm, dram in (("gB", gB4), ("gAT", gAT4), ("uB", uB4),
                             ("uAT", uAT4), ("dB", dB4), ("dAT", dAT4)):
                t = const.tile(list(dram.shape), bf16, name=nm, tag=nm)
                nc.gpsimd.dma_start(t, dram[:])
                fac[nm] = t

            x_sb = const.tile([P, KD, S], bf16, name="x", tag="x")
            # first x quarter leads the scalar ring (needed by job 0);
            # the rest interleaves with the weight stream below.
            nc.scalar.dma_start(x_sb[:, 0:4], x[:, 0:4])

            h_sb = const.tile([P, KF, S], bf16, name="h", tag="h")

            # pass list: (kind, fgroup, n_quads, weight dram, factors, rhs)
            passes = []
            for fg in range(F // FG):
                passes.append(("g", fg, KD // 4, gTp, "gB", "gAT"))
            for fg in range(F // FG):
                passes.append(("u", fg, KD // 4, uTp, "uB", "uAT"))
            for mg in range(D // FG):
                passes.append(("d", mg, KF // 4, dTp, "dB", "dAT"))
            jobs = []
            for pi, ps in enumerate(passes):
                for q in range(ps[2]):
                    jobs.append((pi, q))
            NJ = len(jobs)

            wt_tiles = {}

            def emit_wt(J):
                pi, q = jobs[J]
                kind, fg, nq, wdram, _, _ = passes[pi]
                wt = wpool.tile([P, 4, FG], bf16, name="wt", tag="wt")
                weng = nc.sync if J % 2 == 0 else nc.scalar
                weng.dma_start(wt, wdram[fg * nq + q])
                wt_tiles[J] = wt
                # stream the remaining x quarters down the scalar ring
                # early, between weight transfers (chunks 4q.. needed by
                # gate job q).
                if J in (1, 3, 5):
                    lo = 2 * (J + 1)
                    nc.scalar.dma_start(x_sb[:, lo:lo + 4], x[:, lo:lo + 4])

            sc_tiles = {}

            def emit_sc(J):
                pi, q = jobs[J]
                kind, fg, nq, _, Bn, An = passes[pi]
                sc4 = pssc.tile([P, 4, FG], f32, name="sc", tag="sc")
                B4r, AT4r = fac[Bn], fac[An]
                for i in range(4):
                    nc.tensor.matmul(
                        sc4[:, i],
                        B4r[32 * i:32 * (i + 1), q],
                        AT4r[32 * i:32 * (i + 1), fg * FG:(fg + 1) * FG],
                        start=True, stop=True,
                        tile_position=(32 * i, 0),
                    )
                sc_tiles[J] = sc4

            wr_tiles = {}

            def emit_dq(J):
                wr = rpool.tile([P, 4, FG], bf16, name="wr", tag="wr")
                nc.vector.tensor_mul(out=wr, in0=wt_tiles.pop(J),
                                     in1=sc_tiles.pop(J))
                wr_tiles[J] = wr

            def finish(pi, acc):
                kind, fg, _, _, _, _ = passes[pi]
                if kind == "g":
                    for fi in range(4):
                        nc.scalar.activation(h_sb[:, fg * 4 + fi], acc[fi],
                                             silu)
                elif kind == "u":
                    # psum -> bf16 SBUF on ACT, then a pure-SBUF packed
                    # DVE multiply: keeps DVE free for the dequants.
                    ut = upool.tile([P, 4, S], bf16, name="ut", tag="ut")
                    for fi in range(4):
                        f = fg * 4 + fi
                        nc.scalar.copy(ut[:, fi], acc[fi])
                        nc.vector.tensor_mul(out=h_sb[:, f], in0=h_sb[:, f],
                                             in1=ut[:, fi])
                elif fg < D // FG - 1:
                    ot = opool.tile([P, 4, S], bf16, name="ot", tag="ot")
                    for fi in range(4):
                        nc.scalar.copy(ot[:, fi], acc[fi])
                    weng = nc.sync if fg % 2 == 0 else nc.scalar
                    weng.dma_start(out[fg], ot)
                else:
                    # kernel tail: drain wide — copies split ACT/DVE,
                    # stores split across both HWDGE rings.
                    for half in range(2):
                        ot = opool.tile([P, 2, S], bf16, name="otl", tag="otl")
                        if half == 0:
                            for j in range(2):
                                nc.scalar.copy(ot[:, j], acc[j])
                        else:
                            for j in range(2):
                                nc.vector.tensor_copy(out=ot[:, j],
                                                      in_=acc[2 + j])
                        weng = nc.sync if half == 0 else nc.scalar
                        weng.dma_start(out[fg, :, 2 * half:2 * half + 2], ot)

            DMA_AHEAD = 4
            LOOK = 1
            for J in range(DMA_AHEAD):
                emit_wt(J)
            for J in range(LOOK):
                emit_sc(J)
                emit_dq(J)

            acc = None
            for J, (pi, q) in enumerate(jobs):
                kind, fg, nq, _, _, _ = passes[pi]
                if q == 0:
                    acc = [psacc.tile([P, S], f32, name=f"acc{i}",
                                      tag=f"acc{i}") for i in range(4)]
                if J + DMA_AHEAD < NJ:
                    emit_wt(J + DMA_AHEAD)
                if J + LOOK < NJ:
                    emit_sc(J + LOOK)
                    emit_dq(J + LOOK)
                wr = wr_tiles.pop(J)
                for c in range(4):
                    kc = 4 * q + c
                    rhs = x_sb[:, kc] if kind in "gu" else h_sb[:, kc]
                    for fi in range(4):
                        nc.tensor.matmul(
                            acc[fi],
                            wr[:, c, fi * P:(fi + 1) * P],
                            rhs,
                            start=(q == 0 and c == 0),
                            stop=(q == nq - 1 and c == 3),
                        )
                if q == nq - 1:
                    finish(pi, acc)
    nc.finalize()
    return nc


def _prep_inputs(x, gate_snapped, gate_scale_A, gate_scale_B,
                 up_snapped, up_scale_A, up_scale_B,
                 down_snapped, down_scale_A, down_scale_B):
    bf = ml_dtypes.bfloat16
    x2 = np.ascontiguousarray(
        np.asarray(x, dtype=np.float32).reshape(D, S).astype(bf)
        .reshape(KD, P, S).transpose(1, 0, 2))

    def quad_tile(wT_bf, npass):
        # wT [K, W] bf16 (contraction-major) -> [npass*quads, 128, 4, 512]
        K, W = wT_bf.shape
        nq = K // (4 * P)
        t = wT_bf.reshape(nq, 4, P, npass, FG).transpose(3, 0, 2, 1, 4)
        return np.ascontiguousarray(t.reshape(npass * nq, P, 4, FG))

    def pack_B4(Bmat, nk):
        # B [R, nk*128] -> [128, nk/4, 128]: strip i holds chunks 4q+i
        b = np.asarray(Bmat, dtype=np.float32).reshape(R, nk // 4, 4, P)
        o = np.empty((4 * R, nk // 4, P), dtype=bf)
        for i in range(4):
            o[R * i:R * (i + 1)] = b[:, :, i, :].astype(bf)
        return o

    def pack_AT4(Amat):
        # A [w, R] -> A^T [R, w] replicated on 4 strips -> [128, w]
        at = np.asarray(Amat, dtype=np.float32).T.astype(bf)
        return np.ascontiguousarray(np.concatenate([at] * 4, axis=0))

    gs = np.asarray(gate_snapped, dtype=np.float32)
    us = np.asarray(up_snapped, dtype=np.float32)
    ds = np.asarray(down_snapped, dtype=np.float32)
    gB_f = np.asarray(gate_scale_B, dtype=np.float32)
    uB_f = np.asarray(up_scale_B, dtype=np.float32)
    dB_f = np.asarray(down_scale_B, dtype=np.float32)
    gA_f = np.asarray(gate_scale_A, dtype=np.float32)
    uA_f = np.asarray(up_scale_A, dtype=np.float32)
    dAT4 = pack_AT4(down_scale_A)
    gB4 = pack_B4(gB_f, KD)
    uB4 = pack_B4(uB_f, KD)

    in_maps = []
    for c in range(NCORES):
        lo, hi = c * F, (c + 1) * F
        in_maps.append({
            "x": x2,
            "gTp": quad_tile(gs[lo:hi].T.astype(bf), F // FG),
            "uTp": quad_tile(us[lo:hi].T.astype(bf), F // FG),
            "dTp": quad_tile(ds[:, lo:hi].T.astype(bf), D // FG),
            "gB4": gB4,
            "uB4": uB4,
            "dB4": pack_B4(dB_f[:, lo:hi], KF),
            "gAT4": pack_AT4(gA_f[lo:hi]),
            "uAT4": pack_AT4(uA_f[lo:hi]),
            "dAT4": dAT4,
        })
    return in_maps


def run(trace=False, **inputs):
    if "nc" not in _CACHE:
        _CACHE["nc"] = _build()
    nc = _CACHE["nc"]
    in_maps = _prep_inputs(**inputs)
    try:
        res = run_bass_kernel_spmd(nc, in_maps, list(range(NCORES)), trace=trace)
    except Exception:
        # A transient device flake (NRT_EXEC_UNIT_UNRECOVERABLE) poisons the
        # PJRT client for the process; tearing the backend down and
        # reconnecting recovers it the same way a fresh process does.
        try:
            import jax.extend.backend
            jax.extend.backend.clear_backends()
        except Exception:
            pass
        res = run_bass_kernel_spmd(nc, in_maps, list(range(NCORES)), trace=trace)
    partial = np.zeros((4, P, 4, S), dtype=np.float32)
    for c in range(NCORES):
        partial += res.results[c]["out"].astype(np.float32)
    full = partial.transpose(0, 2, 1, 3).reshape(D, S)
    return full.reshape(1, D, 1, S), res


def kernel(**inputs):
    out, _ = run(trace=False, **inputs)
    return out


if __name__ == "__main__":
    rng = np.random.default_rng(0)
    ins = {
        "x": rng.standard_normal((1, D, 1, S)).astype(np.float32),
        "gate_snapped": (rng.standard_normal((FF, D)) * 0.02).astype(np.float32),
        "gate_scale_A": (rng.standard_normal((FF, R)) * 0.1).astype(np.float32),
        "gate_scale_B": (rng.standard_normal((R, D)) * 0.1).astype(np.float32),
        "up_snapped": (rng.standard_normal((FF, D)) * 0.02).astype(np.float32),
        "up_scale_A": (rng.standard_normal((FF, R)) * 0.1).astype(np.float32),
        "up_scale_B": (rng.standard_normal((R, D)) * 0.1).astype(np.float32),
        "down_snapped": (rng.standard_normal((D, FF)) * 0.02).astype(np.float32),
        "down_scale_A": (rng.standard_normal((D, R)) * 0.1).astype(np.float32),
        "down_scale_B": (rng.standard_normal((R, FF)) * 0.1).astype(np.float32),
    }
    out = kernel(**ins)
    print("kernel ran, out shape", out.shape, "mean abs", np.abs(out).mean())


# revision 8
# speedup vs baseline: 1.1569x; 1.0494x over previous
"""Trainium2 Bass kernel for FFNWithScales (SwiGLU MLP with low-rank dequant scales).

Reference computation (all fp32):
    gate_eff = gate_snapped * (gate_scale_A @ gate_scale_B)       # [8192, 2048]
    up_eff   = up_snapped   * (up_scale_A   @ up_scale_B)         # [8192, 2048]
    down_eff = down_snapped * (down_scale_A @ down_scale_B)       # [2048, 8192]
    h   = silu(gate_eff @ x) * (up_eff @ x)                       # [8192, 512]
    out = down_eff @ h                                            # [2048, 512]

Sharding (8 cores, tensor-parallel on d_ff): core c owns d_ff rows
[c*1024, (c+1)*1024) of gate/up (and the matching columns of down).
Each core computes a full-[2048, 512] partial of the down projection;
bf16 partials are summed in fp32 on the host (the all-reduce step).

Device notes:
  - PE matmul computes psum[M,N] = lhsT[K,M].T @ rhs[K,N] with K on
    partitions. The host ships everything bf16 in final device layout:
    snapped weights are pre-transposed AND pre-tiled so each weight DMA
    is one fully contiguous [128, 4, 512] (512 KiB) "quad" = 4 K-chunks
    of one 512-wide output group. bf16 weights halve the dominant HBM
    stream vs fp32 (measured end-to-end error ~7e-3 of output absmax,
    threshold 2e-2).
  - Work unit = quad job: one 512 KiB weight DMA, one 4-way row-packed
    rank-32 scale matmul (strips at tile_position (32i,0) run
    concurrently in the PE array, output [128,4,512] psum = 4 banks),
    one full-tile DVE dequant multiply (bf16), then 16 [128,128]x
    [128,512] bf16 main matmuls accumulating into 4 psum banks.
    PSUM budget: 4 acc banks + 4 scale banks = all 8.
  - DMA rings: sync+scalar HWDGE alternate the weight stream (and carry
    x / output stores interleaved); the six small scale-factor loads go
    on the gpsimd SWDGE ring so the startup critical path is just
    factors(gpsimd) -> scale mm -> dequant in parallel with wt+x DMAs.
  - Up-projection finish copies psum to bf16 SBUF on ACT, then the
    SwiGLU multiply runs SBUF-only on DVE in 2x packed mode, keeping
    DVE's steady-state load (the dequants) off the critical path at
    pass boundaries.
"""

import numpy as np
import ml_dtypes

import concourse.bass as bass
from concourse import bacc
import concourse.mybir as mybir
from concourse.tile import TileContext
from concourse.bass_utils import run_bass_kernel_spmd

P = 128
D = 2048        # d_model
FF = 8192       # d_ff (global)
S = 512         # sequence
R = 32          # rank
NCORES = 8
F = FF // NCORES          # 1024 local d_ff rows
KD = D // P               # 16 d_model chunks
KF = F // P               # 8 local d_ff chunks
FG = 512                  # free-dim group (psum bank width)

f32 = mybir.dt.float32
bf16 = mybir.dt.bfloat16

_CACHE = {}


def _build():
    nc = bacc.Bacc()
    x = nc.declare_dram_parameter("x", [P, KD, S], bf16, isOutput=False)
    # quad-tiled weights: [pass*quads, 128, 4 k-chunks, 512 out-cols]
    gTp = nc.declare_dram_parameter("gTp", [8, P, 4, FG], bf16, isOutput=False)
    uTp = nc.declare_dram_parameter("uTp", [8, P, 4, FG], bf16, isOutput=False)
    dTp = nc.declare_dram_parameter("dTp", [8, P, 4, FG], bf16, isOutput=False)
    # B4 [128, nk/4, 128]: strip i (partitions 32i..32i+31) holds scale-B
    # cols for chunk 4q+i; AT4 [128, w]: A^T replicated on all 4 strips.
    gB4 = nc.declare_dram_parameter("gB4", [P, KD // 4, P], bf16, isOutput=False)
    uB4 = nc.declare_dram_parameter("uB4", [P, KD // 4, P], bf16, isOutput=False)
    dB4 = nc.declare_dram_parameter("dB4", [P, KF // 4, P], bf16, isOutput=False)
    gAT4 = nc.declare_dram_parameter("gAT4", [P, F], bf16, isOutput=False)
    uAT4 = nc.declare_dram_parameter("uAT4", [P, F], bf16, isOutput=False)
    dAT4 = nc.declare_dram_parameter("dAT4", [P, D], bf16, isOutput=False)
    out = nc.declare_dram_parameter("out", [4, P, 4, S], bf16, isOutput=True)

    silu = mybir.ActivationFunctionType.Silu

    with TileContext(nc) as tc:
        with (
            tc.tile_pool(name="const", bufs=1) as const,
            tc.tile_pool(name="wstream", bufs=5) as wpool,
            tc.tile_pool(name="wready", bufs=3) as rpool,
            tc.tile_pool(name="utmp", bufs=2) as upool,
            tc.tile_pool(name="obuf", bufs=2) as opool,
            tc.tile_pool(name="psacc", bufs=1, space="PSUM") as psacc,
            tc.tile_pool(name="pssc", bufs=1, space="PSUM") as pssc,
        ):
            # PE warm-up: HAM keeps the PE clock-gated at 1.2 GHz until it
            # sees ~3.4us of sustained activity, so burn the initial DMA
            # wait on dummy matmuls over a zeroed tile (result never read;
            # the psum tag is recycled by the first real accumulator).
            zt = const.tile([P, 640], bf16, name="zt", tag="zt")
            nc.gpsimd.memset(zt, 0)
            warm = psacc.tile([P, S], f32, name="warm", tag="acc0")
            for _ in range(8):
                nc.tensor.matmul(warm, zt[:, 0:P], zt[:, P:P + S],
                                 start=True, stop=True)

            # Factor loads ride the fast HWDGE rings: the gate factors lead
            # the sync ring (they gate the first scale matmul); up/down
            # factors interleave with x on the scalar ring (needed much
            # later). gAT second half follows the first weight quad.
            fac = {}

            def load_fac(nm, dram, eng):
                t = const.tile(list(dram.shape), bf16, name=nm, tag=nm)
                if eng is not None:
                    eng.dma_start(t, dram[:])
                fac[nm] = t

            load_fac("gB", gB4, nc.sync)
            load_fac("gAT", gAT4, None)
            nc.sync.dma_start(fac["gAT"][:, 0:FG], gAT4[:, 0:FG])
            load_fac("uB", uB4, None)
            load_fac("uAT", uAT4, None)
            load_fac("dB", dB4, None)
            load_fac("dAT", dAT4, None)

            x_sb = const.tile([P, KD, S], bf16, name="x", tag="x")
            # first x quarter leads the scalar ring (needed by job 0);
            # the rest interleaves with the weight stream below.
            nc.scalar.dma_start(x_sb[:, 0:4], x[:, 0:4])

            h_sb = const.tile([P, KF, S], bf16, name="h", tag="h")

            # pass list: (kind, fgroup, n_quads, weight dram, factors, rhs)
            passes = []
            for fg in range(F // FG):
                passes.append(("g", fg, KD // 4, gTp, "gB", "gAT"))
            for fg in range(F // FG):
                passes.append(("u", fg, KD // 4, uTp, "uB", "uAT"))
            for mg in range(D // FG):
                passes.append(("d", mg, KF // 4, dTp, "dB", "dAT"))
            jobs = []
            for pi, ps in enumerate(passes):
                for q in range(ps[2]):
                    jobs.append((pi, q))
            NJ = len(jobs)

            wt_tiles = {}

            def emit_wt(J):
                pi, q = jobs[J]
                kind, fg, nq, wdram, _, _ = passes[pi]
                wt = wpool.tile([P, 4, FG], bf16, name="wt", tag="wt")
                weng = nc.sync if J % 2 == 0 else nc.scalar
                if J == 0:
                    # split the first quad per chunk so the startup chain
                    # (factors -> sc -> chunk dequant -> mains) starts on
                    # chunk 0 without waiting for the full 512 KiB.
                    for c in range(4):
                        nc.sync.dma_start(wt[:, c], wdram[0, :, c])
                    # gAT second half follows immediately behind
                    nc.sync.dma_start(fac["gAT"][:, FG:2 * FG],
                                      gAT4[:, FG:2 * FG])
                else:
                    weng.dma_start(wt, wdram[fg * nq + q])
                wt_tiles[J] = wt
                # stream the remaining x quarters and the up/down factors
                # down the scalar ring early, between weight transfers
                # (x chunks 4q.. are needed by gate job q).
                if J in (1, 3, 5):
                    lo = 2 * (J + 1)
                    nc.scalar.dma_start(x_sb[:, lo:lo + 4], x[:, lo:lo + 4])
                elif J == 7:
                    nc.scalar.dma_start(fac["uB"], uB4[:])
                    nc.scalar.dma_start(fac["uAT"], uAT4[:])
                elif J == 9:
                    nc.scalar.dma_start(fac["dB"], dB4[:])
                    nc.scalar.dma_start(fac["dAT"], dAT4[:])

            sc_tiles = {}

            def emit_sc(J):
                pi, q = jobs[J]
                kind, fg, nq, _, Bn, An = passes[pi]
                sc4 = pssc.tile([P, 4, FG], f32, name="sc", tag="sc")
                B4r, AT4r = fac[Bn], fac[An]
                for i in range(4):
                    nc.tensor.matmul(
                        sc4[:, i],
                        B4r[32 * i:32 * (i + 1), q],
                        AT4r[32 * i:32 * (i + 1), fg * FG:(fg + 1) * FG],
                        start=True, stop=True,
                        tile_position=(32 * i, 0),
                    )
                sc_tiles[J] = sc4

            wr_tiles = {}

            def emit_dq(J):
                wr = rpool.tile([P, 4, FG], bf16, name="wr", tag="wr")
                wt, sc4 = wt_tiles.pop(J), sc_tiles.pop(J)
                if J == 0:
                    # per-chunk at startup so mains begin after chunk 0
                    for c in range(4):
                        nc.vector.tensor_mul(out=wr[:, c], in0=wt[:, c],
                                             in1=sc4[:, c])
                else:
                    nc.vector.tensor_mul(out=wr, in0=wt, in1=sc4)
                wr_tiles[J] = wr

            def finish(pi, acc):
                kind, fg, _, _, _, _ = passes[pi]
                if kind == "g":
                    for fi in range(4):
                        nc.scalar.activation(h_sb[:, fg * 4 + fi], acc[fi],
                                             silu)
                elif kind == "u":
                    # psum -> bf16 SBUF on ACT, then a pure-SBUF packed
                    # DVE multiply: keeps DVE free for the dequants.
                    ut = upool.tile([P, 4, S], bf16, name="ut", tag="ut")
                    for fi in range(4):
                        f = fg * 4 + fi
                        nc.scalar.copy(ut[:, fi], acc[fi])
                        nc.vector.tensor_mul(out=h_sb[:, f], in0=h_sb[:, f],
                                             in1=ut[:, fi])
                else:
                    ot = opool.tile([P, 4, S], bf16, name="ot", tag="ot")
                    for fi in range(4):
                        nc.scalar.copy(ot[:, fi], acc[fi])
                    weng = nc.sync if fg % 2 == 0 else nc.scalar
                    weng.dma_start(out[fg], ot)

            DMA_AHEAD = 4
            LOOK = 1
            for J in range(DMA_AHEAD):
                emit_wt(J)
            for J in range(LOOK):
                emit_sc(J)
                emit_dq(J)

            acc = None
            for J, (pi, q) in enumerate(jobs):
                kind, fg, nq, _, _, _ = passes[pi]
                if q == 0:
                    acc = [psacc.tile([P, S], f32, name=f"acc{i}",
                                      tag=f"acc{i}") for i in range(4)]
                if J + DMA_AHEAD < NJ:
                    emit_wt(J + DMA_AHEAD)
                if J + LOOK < NJ:
                    emit_sc(J + LOOK)
                    emit_dq(J + LOOK)
                wr = wr_tiles.pop(J)
                if J == NJ - 1:
                    # kernel tail: fi-major so each output bank finishes 4
                    # matmuls apart — copies and stores overlap the last
                    # matmuls instead of serializing after them. Copies
                    # split ACT/DVE, stores split across both HWDGE rings.
                    for fi in range(4):
                        for c in range(4):
                            nc.tensor.matmul(
                                acc[fi],
                                wr[:, c, fi * P:(fi + 1) * P],
                                h_sb[:, 4 * q + c],
                                start=False,
                                stop=(c == 3),
                            )
                        ot = opool.tile([P, S], bf16, name="otl",
                                        tag=f"otl{fi}")
                        if fi % 2 == 0:
                            nc.scalar.copy(ot, acc[fi])
                        else:
                            nc.vector.tensor_copy(out=ot, in_=acc[fi])
                        weng = nc.sync if fi % 2 == 0 else nc.scalar
                        weng.dma_start(out[fg, :, fi], ot)
                    continue
                for c in range(4):
                    kc = 4 * q + c
                    rhs = x_sb[:, kc] if kind in "gu" else h_sb[:, kc]
                    for fi in range(4):
                        nc.tensor.matmul(
                            acc[fi],
                            wr[:, c, fi * P:(fi + 1) * P],
                            rhs,
                            start=(q == 0 and c == 0),
                            stop=(q == nq - 1 and c == 3),
                        )
                if q == nq - 1:
                    finish(pi, acc)
    nc.finalize()
    return nc


def _prep_inputs(x, gate_snapped, gate_scale_A, gate_scale_B,
                 up_snapped, up_scale_A, up_scale_B,
                 down_snapped, down_scale_A, down_scale_B):
    bf = ml_dtypes.bfloat16
    x2 = np.ascontiguousarray(
        np.asarray(x, dtype=np.float32).reshape(D, S).astype(bf)
        .reshape(KD, P, S).transpose(1, 0, 2))

    def quad_tile(wT_bf, npass):
        # wT [K, W] bf16 (contraction-major) -> [npass*quads, 128, 4, 512]
        K, W = wT_bf.shape
        nq = K // (4 * P)
        t = wT_bf.reshape(nq, 4, P, npass, FG).transpose(3, 0, 2, 1, 4)
        return np.ascontiguousarray(t.reshape(npass * nq, P, 4, FG))

    def pack_B4(Bmat, nk):
        # B [R, nk*128] -> [128, nk/4, 128]: strip i holds chunks 4q+i
        b = np.asarray(Bmat, dtype=np.float32).reshape(R, nk // 4, 4, P)
        o = np.empty((4 * R, nk // 4, P), dtype=bf)
        for i in range(4):
            o[R * i:R * (i + 1)] = b[:, :, i, :].astype(bf)
        return o

    def pack_AT4(Amat):
        # A [w, R] -> A^T [R, w] replicated on 4 strips -> [128, w]
        at = np.asarray(Amat, dtype=np.float32).T.astype(bf)
        return np.ascontiguousarray(np.concatenate([at] * 4, axis=0))

    gs = np.asarray(gate_snapped, dtype=np.float32)
    us = np.asarray(up_snapped, dtype=np.float32)
    ds = np.asarray(down_snapped, dtype=np.float32)
    gB_f = np.asarray(gate_scale_B, dtype=np.float32)
    uB_f = np.asarray(up_scale_B, dtype=np.float32)
    dB_f = np.asarray(down_scale_B, dtype=np.float32)
    gA_f = np.asarray(gate_scale_A, dtype=np.float32)
    uA_f = np.asarray(up_scale_A, dtype=np.float32)
    dAT4 = pack_AT4(down_scale_A)
    gB4 = pack_B4(gB_f, KD)
    uB4 = pack_B4(uB_f, KD)

    in_maps = []
    for c in range(NCORES):
        lo, hi = c * F, (c + 1) * F
        in_maps.append({
            "x": x2,
            "gTp": quad_tile(gs[lo:hi].T.astype(bf), F // FG),
            "uTp": quad_tile(us[lo:hi].T.astype(bf), F // FG),
            "dTp": quad_tile(ds[:, lo:hi].T.astype(bf), D // FG),
            "gB4": gB4,
            "uB4": uB4,
            "dB4": pack_B4(dB_f[:, lo:hi], KF),
            "gAT4": pack_AT4(gA_f[lo:hi]),
            "uAT4": pack_AT4(uA_f[lo:hi]),
            "dAT4": dAT4,
        })
    return in_maps


def run(trace=False, **inputs):
    if "nc" not in _CACHE:
        _CACHE["nc"] = _build()
    nc = _CACHE["nc"]
    in_maps = _prep_inputs(**inputs)
    try:
        res = run_bass_kernel_spmd(nc, in_maps, list(range(NCORES)), trace=trace)
    except Exception:
        # A transient device flake (NRT_EXEC_UNIT_UNRECOVERABLE) poisons the
        # PJRT client for the process; tearing the backend down and
        # reconnecting recovers it the same way a fresh process does.
        try:
            import jax.extend.backend
            jax.extend.backend.clear_backends()
        except Exception:
            pass
        res = run_bass_kernel_spmd(nc, in_maps, list(range(NCORES)), trace=trace)
    partial = np.zeros((4, P, 4, S), dtype=np.float32)
    for c in range(NCORES):
        partial += res.results[c]["out"].astype(np.float32)
    full = partial.transpose(0, 2, 1, 3).reshape(D, S)
    return full.reshape(1, D, 1, S), res


def kernel(**inputs):
    out, _ = run(trace=False, **inputs)
    return out


if __name__ == "__main__":
    rng = np.random.default_rng(0)
    ins = {
        "x": rng.standard_normal((1, D, 1, S)).astype(np.float32),
        "gate_snapped": (rng.standard_normal((FF, D)) * 0.02).astype(np.float32),
        "gate_scale_A": (rng.standard_normal((FF, R)) * 0.1).astype(np.float32),
        "gate_scale_B": (rng.standard_normal((R, D)) * 0.1).astype(np.float32),
        "up_snapped": (rng.standard_normal((FF, D)) * 0.02).astype(np.float32),
        "up_scale_A": (rng.standard_normal((FF, R)) * 0.1).astype(np.float32),
        "up_scale_B": (rng.standard_normal((R, D)) * 0.1).astype(np.float32),
        "down_snapped": (rng.standard_normal((D, FF)) * 0.02).astype(np.float32),
        "down_scale_A": (rng.standard_normal((D, R)) * 0.1).astype(np.float32),
        "down_scale_B": (rng.standard_normal((R, FF)) * 0.1).astype(np.float32),
    }
    out = kernel(**ins)
    print("kernel ran, out shape", out.shape, "mean abs", np.abs(out).mean())


# revision 20
# speedup vs baseline: 1.1596x; 1.0023x over previous
"""Trainium2 Bass kernel for FFNWithScales (SwiGLU MLP with low-rank dequant scales).

Reference computation (all fp32):
    gate_eff = gate_snapped * (gate_scale_A @ gate_scale_B)       # [8192, 2048]
    up_eff   = up_snapped   * (up_scale_A   @ up_scale_B)         # [8192, 2048]
    down_eff = down_snapped * (down_scale_A @ down_scale_B)       # [2048, 8192]
    h   = silu(gate_eff @ x) * (up_eff @ x)                       # [8192, 512]
    out = down_eff @ h                                            # [2048, 512]

Sharding (8 cores, tensor-parallel on d_ff): core c owns d_ff rows
[c*1024, (c+1)*1024) of gate/up (and the matching columns of down).
Each core computes a full-[2048, 512] partial of the down projection;
bf16 partials are summed in fp32 on the host (the all-reduce step).

Device notes:
  - PE matmul computes psum[M,N] = lhsT[K,M].T @ rhs[K,N] with K on
    partitions. The host ships everything bf16 in final device layout:
    snapped weights are pre-transposed AND pre-tiled so each weight DMA
    is one fully contiguous [128, 4, 512] (512 KiB) "quad" = 4 K-chunks
    of one 512-wide output group. bf16 weights halve the dominant HBM
    stream vs fp32 (measured end-to-end error ~7e-3 of output absmax,
    threshold 2e-2).
  - Work unit = quad job: one 512 KiB weight DMA, one 4-way row-packed
    rank-32 scale matmul (strips at tile_position (32i,0) run
    concurrently in the PE array, output [128,4,512] psum = 4 banks),
    one full-tile DVE dequant multiply (bf16), then 16 [128,128]x
    [128,512] bf16 main matmuls accumulating into 4 psum banks.
    PSUM budget: 4 acc banks + 4 scale banks = all 8.
  - DMA rings: sync+scalar HWDGE alternate the weight stream (and carry
    x / output stores interleaved); the six small scale-factor loads go
    on the gpsimd SWDGE ring so the startup critical path is just
    factors(gpsimd) -> scale mm -> dequant in parallel with wt+x DMAs.
  - Up-projection finish copies psum to bf16 SBUF on ACT, then the
    SwiGLU multiply runs SBUF-only on DVE in 2x packed mode, keeping
    DVE's steady-state load (the dequants) off the critical path at
    pass boundaries.
"""

import numpy as np
import ml_dtypes

import concourse.bass as bass
from concourse import bacc
import concourse.mybir as mybir
from concourse.tile import TileContext
from concourse.bass_utils import run_bass_kernel_spmd

P = 128
D = 2048        # d_model
FF = 8192       # d_ff (global)
S = 512         # sequence
R = 32          # rank
NCORES = 8
F = FF // NCORES          # 1024 local d_ff rows
KD = D // P               # 16 d_model chunks
KF = F // P               # 8 local d_ff chunks
FG = 512                  # free-dim group (psum bank width)

f32 = mybir.dt.float32
bf16 = mybir.dt.bfloat16

_CACHE = {}


def _build():
    nc = bacc.Bacc()
    x = nc.declare_dram_parameter("x", [P, KD, S], bf16, isOutput=False)
    # quad-tiled weights: [pass*quads, 128, 4 k-chunks, 512 out-cols]
    gTp = nc.declare_dram_parameter("gTp", [8, P, 4, FG], bf16, isOutput=False)
    uTp = nc.declare_dram_parameter("uTp", [8, P, 4, FG], bf16, isOutput=False)
    dTp = nc.declare_dram_parameter("dTp", [8, P, 4, FG], bf16, isOutput=False)
    # Packed per-matrix scale factors, one DMA each: cols [0, nk*32) hold
    # B strips (strip i = partitions 32i..32i+31, chunk 4q+i at col q*128),
    # the rest holds A^T replicated on all 4 strips.
    gFac = nc.declare_dram_parameter("gFac", [P, KD * R + F], bf16, isOutput=False)
    uFac = nc.declare_dram_parameter("uFac", [P, KD * R + F], bf16, isOutput=False)
    dFac = nc.declare_dram_parameter("dFac", [P, KF * R + D], bf16, isOutput=False)
    out = nc.declare_dram_parameter("out", [4, P, 4, S], bf16, isOutput=True)

    silu = mybir.ActivationFunctionType.Silu

    with TileContext(nc) as tc:
        with (
            tc.tile_pool(name="const", bufs=1) as const,
            tc.tile_pool(name="wstream", bufs=6) as wpool,
            tc.tile_pool(name="wready", bufs=3) as rpool,
            tc.tile_pool(name="utmp", bufs=2) as upool,
            tc.tile_pool(name="obuf", bufs=2) as opool,
            tc.tile_pool(name="psacc", bufs=1, space="PSUM") as psacc,
            tc.tile_pool(name="pssc", bufs=1, space="PSUM") as pssc,
        ):
            # PE warm-up: HAM keeps the PE clock-gated at 1.2 GHz until it
            # sees ~3.4us of sustained activity, so burn the initial DMA
            # wait on dummy matmuls over a zeroed tile (result never read;
            # the psum tag is recycled by the first real accumulator).
            zt = const.tile([P, 640], bf16, name="zt", tag="zt")
            nc.gpsimd.memset(zt, 0)
            warm = psacc.tile([P, S], f32, name="warm", tag="acc0")
            for _ in range(8):
                nc.tensor.matmul(warm, zt[:, 0:P], zt[:, P:P + S],
                                 start=True, stop=True)

            # Factor loads ride the fast HWDGE rings, one packed DMA per
            # matrix (DMA issue costs ~650ns of engine time and completion
            # ~2us of latency, so fewer+bigger wins at startup): gate
            # factors lead the scalar ring in parallel with the first
            # weight quad on sync; up/down factors interleave with the
            # weight stream later (needed ~30us in).
            fac = {}

            def load_fac(nm, dram, eng):
                t = const.tile(list(dram.shape), bf16, name=nm, tag=nm)
                if eng is not None:
                    eng.dma_start(t, dram[:])
                fac[nm] = t

            load_fac("gF", gFac, nc.scalar)
            load_fac("uF", uFac, None)
            load_fac("dF", dFac, None)

            x_sb = const.tile([P, KD, S], bf16, name="x", tag="x")
            # first x quarter next on the scalar ring (needed by job 0);
            # the rest interleaves with the weight stream below.
            nc.scalar.dma_start(x_sb[:, 0:4], x[:, 0:4])

            h_sb = const.tile([P, KF, S], bf16, name="h", tag="h")

            # pass list: (kind, fgroup, n_quads, weight dram, factors)
            passes = []
            for fg in range(F // FG):
                passes.append(("g", fg, KD // 4, gTp, "gF"))
            for fg in range(F // FG):
                passes.append(("u", fg, KD // 4, uTp, "uF"))
            for mg in range(D // FG):
                passes.append(("d", mg, KF // 4, dTp, "dF"))
            jobs = []
            for pi, ps in enumerate(passes):
                for q in range(ps[2]):
                    jobs.append((pi, q))
            NJ = len(jobs)

            wt_tiles = {}

            def emit_wt(J):
                pi, q = jobs[J]
                kind, fg, nq, wdram, _ = passes[pi]
                wt = wpool.tile([P, 4, FG], bf16, name="wt", tag="wt")
                weng = nc.sync if J % 2 == 0 else nc.scalar
                weng.dma_start(wt, wdram[fg * nq + q])
                wt_tiles[J] = wt
                # stream the remaining x quarters and the up/down factors
                # down the scalar ring early, between weight transfers
                # (x chunks 4q.. are needed by gate job q).
                if J in (1, 3, 5):
                    lo = 2 * (J + 1)
                    nc.scalar.dma_start(x_sb[:, lo:lo + 4], x[:, lo:lo + 4])
                elif J == 7:
                    nc.scalar.dma_start(fac["uF"], uFac[:])
                elif J == 9:
                    nc.scalar.dma_start(fac["dF"], dFac[:])

            sc_tiles = {}

            def emit_sc(J):
                pi, q = jobs[J]
                kind, fg, nq, _, Fn = passes[pi]
                sc4 = pssc.tile([P, 4, FG], f32, name="sc", tag="sc")
                Ft = fac[Fn]
                aoff = nq * P + fg * FG  # A^T block starts after B strips
                for i in range(4):
                    nc.tensor.matmul(
                        sc4[:, i],
                        Ft[32 * i:32 * (i + 1), q * P:(q + 1) * P],
                        Ft[32 * i:32 * (i + 1), aoff:aoff + FG],
                        start=True, stop=True,
                        tile_position=(32 * i, 0),
                    )
                sc_tiles[J] = sc4

            wr_tiles = {}

            def emit_dq(J):
                wr = rpool.tile([P, 4, FG], bf16, name="wr", tag="wr")
                nc.vector.tensor_mul(out=wr, in0=wt_tiles.pop(J),
                                     in1=sc_tiles.pop(J))
                wr_tiles[J] = wr

            def finish(pi, acc):
                kind, fg = passes[pi][0], passes[pi][1]
                if kind == "g":
                    for fi in range(4):
                        nc.scalar.activation(h_sb[:, fg * 4 + fi], acc[fi],
                                             silu)
                elif kind == "u":
                    # psum -> bf16 SBUF on ACT, then a pure-SBUF packed
                    # DVE multiply: keeps DVE free for the dequants.
                    ut = upool.tile([P, 4, S], bf16, name="ut", tag="ut")
                    for fi in range(4):
                        f = fg * 4 + fi
                        nc.scalar.copy(ut[:, fi], acc[fi])
                        nc.vector.tensor_mul(out=h_sb[:, f], in0=h_sb[:, f],
                                             in1=ut[:, fi])
                else:
                    ot = opool.tile([P, 4, S], bf16, name="ot", tag="ot")
                    for fi in range(4):
                        nc.scalar.copy(ot[:, fi], acc[fi])
                    weng = nc.sync if fg % 2 == 0 else nc.scalar
                    weng.dma_start(out[fg], ot)

            DMA_AHEAD = 5
            LOOK = 1
            for J in range(DMA_AHEAD):
                emit_wt(J)
            for J in range(LOOK):
                emit_sc(J)
                emit_dq(J)

            acc = None
            for J, (pi, q) in enumerate(jobs):
                kind, fg, nq = passes[pi][0], passes[pi][1], passes[pi][2]
                if q == 0:
                    acc = [psacc.tile([P, S], f32, name=f"acc{i}",
                                      tag=f"acc{i}") for i in range(4)]
                if J + DMA_AHEAD < NJ:
                    emit_wt(J + DMA_AHEAD)
                if J + LOOK < NJ:
                    emit_sc(J + LOOK)
                    emit_dq(J + LOOK)
                wr = wr_tiles.pop(J)
                if J == NJ - 1:
                    # kernel tail: fi-major so each output bank finishes 4
                    # matmuls apart — copies and stores overlap the last
                    # matmuls instead of serializing after them. Copies
                    # split ACT/DVE, stores split across both HWDGE rings.
                    for fi in range(4):
                        for c in range(4):
                            nc.tensor.matmul(
                                acc[fi],
                                wr[:, c, fi * P:(fi + 1) * P],
                                h_sb[:, 4 * q + c],
                                start=False,
                                stop=(c == 3),
                            )
                        ot = opool.tile([P, S], bf16, name="otl",
                                        tag=f"otl{fi}")
                        if fi % 2 == 0:
                            nc.scalar.copy(ot, acc[fi])
                        else:
                            nc.vector.tensor_copy(out=ot, in_=acc[fi])
                        weng = nc.sync if fi % 2 == 0 else nc.scalar
                        weng.dma_start(out[fg, :, fi], ot)
                    continue
                for c in range(4):
                    kc = 4 * q + c
                    rhs = x_sb[:, kc] if kind in "gu" else h_sb[:, kc]
                    for fi in range(4):
                        nc.tensor.matmul(
                            acc[fi],
                            wr[:, c, fi * P:(fi + 1) * P],
                            rhs,
                            start=(q == 0 and c == 0),
                            stop=(q == nq - 1 and c == 3),
                        )
                if q == nq - 1:
                    finish(pi, acc)
    nc.finalize()
    return nc


def _prep_inputs(x, gate_snapped, gate_scale_A, gate_scale_B,
                 up_snapped, up_scale_A, up_scale_B,
                 down_snapped, down_scale_A, down_scale_B):
    bf = ml_dtypes.bfloat16
    x2 = np.ascontiguousarray(
        np.asarray(x, dtype=np.float32).reshape(D, S).astype(bf)
        .reshape(KD, P, S).transpose(1, 0, 2))

    def quad_tile(wT_bf, npass):
        # wT [K, W] bf16 (contraction-major) -> [npass*quads, 128, 4, 512]
        K, W = wT_bf.shape
        nq = K // (4 * P)
        t = wT_bf.reshape(nq, 4, P, npass, FG).transpose(3, 0, 2, 1, 4)
        return np.ascontiguousarray(t.reshape(npass * nq, P, 4, FG))

    def pack_fac(Bmat, Amat, nk):
        # one packed [128, nk*32 + w] factor block per matrix:
        # cols [0, nk*32): B strips (strip i = rows 32i.., chunk 4q+i at
        # col-group q); cols [nk*32, ..): A^T replicated on all 4 strips.
        b = np.asarray(Bmat, dtype=np.float32).reshape(R, nk // 4, 4, P)
        at = np.asarray(Amat, dtype=np.float32).T.astype(bf)  # [R, w]
        w = at.shape[1]
        o = np.empty((4 * R, nk // 4 * P + w), dtype=bf)
        for i in range(4):
            o[R * i:R * (i + 1), :nk // 4 * P] = \
                b[:, :, i, :].astype(bf).reshape(R, nk // 4 * P)
            o[R * i:R * (i + 1), nk // 4 * P:] = at
        return o

    gs = np.asarray(gate_snapped, dtype=np.float32)
    us = np.asarray(up_snapped, dtype=np.float32)
    ds = np.asarray(down_snapped, dtype=np.float32)
    gB_f = np.asarray(gate_scale_B, dtype=np.float32)
    uB_f = np.asarray(up_scale_B, dtype=np.float32)
    dB_f = np.asarray(down_scale_B, dtype=np.float32)
    gA_f = np.asarray(gate_scale_A, dtype=np.float32)
    uA_f = np.asarray(up_scale_A, dtype=np.float32)
    dA_f = np.asarray(down_scale_A, dtype=np.float32)

    in_maps = []
    for c in range(NCORES):
        lo, hi = c * F, (c + 1) * F
        in_maps.append({
            "x": x2,
            "gTp": quad_tile(gs[lo:hi].T.astype(bf), F // FG),
            "uTp": quad_tile(us[lo:hi].T.astype(bf), F // FG),
            "dTp": quad_tile(ds[:, lo:hi].T.astype(bf), D // FG),
            "gFac": pack_fac(gB_f, gA_f[lo:hi], KD),
            "uFac": pack_fac(uB_f, uA_f[lo:hi], KD),
            "dFac": pack_fac(dB_f[:, lo:hi], dA_f, KF),
        })
    return in_maps


def run(trace=False, **inputs):
    if "nc" not in _CACHE:
        _CACHE["nc"] = _build()
    nc = _CACHE["nc"]
    in_maps = _prep_inputs(**inputs)
    try:
        res = run_bass_kernel_spmd(nc, in_maps, list(range(NCORES)), trace=trace)
    except Exception:
        # A transient device flake (NRT_EXEC_UNIT_UNRECOVERABLE) poisons the
        # PJRT client for the process; tearing the backend down and
        # reconnecting recovers it the same way a fresh process does.
        try:
            import jax.extend.backend
            jax.extend.backend.clear_backends()
        except Exception:
            pass
        res = run_bass_kernel_spmd(nc, in_maps, list(range(NCORES)), trace=trace)
    partial = np.zeros((4, P, 4, S), dtype=np.float32)
    for c in range(NCORES):
        partial += res.results[c]["out"].astype(np.float32)
    full = partial.transpose(0, 2, 1, 3).reshape(D, S)
    return full.reshape(1, D, 1, S), res


def kernel(**inputs):
    out, _ = run(trace=False, **inputs)
    return out


if __name__ == "__main__":
    rng = np.random.default_rng(0)
    ins = {
        "x": rng.standard_normal((1, D, 1, S)).astype(np.float32),
        "gate_snapped": (rng.standard_normal((FF, D)) * 0.02).astype(np.float32),
        "gate_scale_A": (rng.standard_normal((FF, R)) * 0.1).astype(np.float32),
        "gate_scale_B": (rng.standard_normal((R, D)) * 0.1).astype(np.float32),
        "up_snapped": (rng.standard_normal((FF, D)) * 0.02).astype(np.float32),
        "up_scale_A": (rng.standard_normal((FF, R)) * 0.1).astype(np.float32),
        "up_scale_B": (rng.standard_normal((R, D)) * 0.1).astype(np.float32),
        "down_snapped": (rng.standard_normal((D, FF)) * 0.02).astype(np.float32),
        "down_scale_A": (rng.standard_normal((D, R)) * 0.1).astype(np.float32),
        "down_scale_B": (rng.standard_normal((R, FF)) * 0.1).astype(np.float32),
    }
    out = kernel(**ins)
    print("kernel ran, out shape", out.shape, "mean abs", np.abs(out).mean())


# revision 27
# speedup vs baseline: 1.1639x; 1.0037x over previous
"""Trainium2 Bass kernel for FFNWithScales (SwiGLU MLP with low-rank dequant scales).

Reference computation (all fp32):
    gate_eff = gate_snapped * (gate_scale_A @ gate_scale_B)       # [8192, 2048]
    up_eff   = up_snapped   * (up_scale_A   @ up_scale_B)         # [8192, 2048]
    down_eff = down_snapped * (down_scale_A @ down_scale_B)       # [2048, 8192]
    h   = silu(gate_eff @ x) * (up_eff @ x)                       # [8192, 512]
    out = down_eff @ h                                            # [2048, 512]

Sharding (8 cores, tensor-parallel on d_ff): core c owns d_ff rows
[c*1024, (c+1)*1024) of gate/up (and the matching columns of down).
Each core computes a full-[2048, 512] partial of the down projection;
bf16 partials are summed in fp32 on the host (the all-reduce step).

Device notes:
  - PE matmul computes psum[M,N] = lhsT[K,M].T @ rhs[K,N] with K on
    partitions. The host ships everything bf16 in final device layout:
    snapped weights are pre-transposed AND pre-tiled so each weight DMA
    is one fully contiguous [128, 4, 512] (512 KiB) "quad" = 4 K-chunks
    of one 512-wide output group. bf16 weights halve the dominant HBM
    stream vs fp32 (measured end-to-end error ~7e-3 of output absmax,
    threshold 2e-2).
  - Work unit = quad job: one 512 KiB weight DMA, one 4-way row-packed
    rank-32 scale matmul (strips at tile_position (32i,0) run
    concurrently in the PE array, output [128,4,512] psum = 4 banks),
    one full-tile DVE dequant multiply (bf16), then 16 [128,128]x
    [128,512] bf16 main matmuls accumulating into 4 psum banks.
    PSUM budget: 4 acc banks + 4 scale banks = all 8.
  - DMA rings: sync+scalar HWDGE alternate the weight stream (and carry
    x / output stores interleaved); the six small scale-factor loads go
    on the gpsimd SWDGE ring so the startup critical path is just
    factors(gpsimd) -> scale mm -> dequant in parallel with wt+x DMAs.
  - Up-projection finish copies psum to bf16 SBUF on ACT, then the
    SwiGLU multiply runs SBUF-only on DVE in 2x packed mode, keeping
    DVE's steady-state load (the dequants) off the critical path at
    pass boundaries.
"""

import numpy as np
import ml_dtypes

import concourse.bass as bass
from concourse import bacc
import concourse.mybir as mybir
from concourse.tile import TileContext
from concourse.bass_utils import run_bass_kernel_spmd

P = 128
D = 2048        # d_model
FF = 8192       # d_ff (global)
S = 512         # sequence
R = 32          # rank
NCORES = 8
F = FF // NCORES          # 1024 local d_ff rows
KD = D // P               # 16 d_model chunks
KF = F // P               # 8 local d_ff chunks
FG = 512                  # free-dim group (psum bank width)

f32 = mybir.dt.float32
bf16 = mybir.dt.bfloat16

_CACHE = {}


def _build():
    nc = bacc.Bacc()
    x = nc.declare_dram_parameter("x", [P, KD, S], bf16, isOutput=False)
    # quad-tiled weights: [pass*quads, 128, 4 k-chunks, 512 out-cols]
    gTp = nc.declare_dram_parameter("gTp", [8, P, 4, FG], bf16, isOutput=False)
    uTp = nc.declare_dram_parameter("uTp", [8, P, 4, FG], bf16, isOutput=False)
    dTp = nc.declare_dram_parameter("dTp", [8, P, 4, FG], bf16, isOutput=False)
    # Packed per-matrix scale factors, one DMA each: cols [0, nk*32) hold
    # B strips (strip i = partitions 32i..32i+31, chunk 4q+i at col q*128),
    # the rest holds A^T replicated on all 4 strips.
    gFac = nc.declare_dram_parameter("gFac", [P, KD * R + F], bf16, isOutput=False)
    uFac = nc.declare_dram_parameter("uFac", [P, KD * R + F], bf16, isOutput=False)
    dFac = nc.declare_dram_parameter("dFac", [P, KF * R + D], bf16, isOutput=False)
    out = nc.declare_dram_parameter("out", [4, P, 4, S], bf16, isOutput=True)

    silu = mybir.ActivationFunctionType.Silu

    with TileContext(nc) as tc:
        with (
            tc.tile_pool(name="const", bufs=1) as const,
            tc.tile_pool(name="wstream", bufs=6) as wpool,
            tc.tile_pool(name="wready", bufs=3) as rpool,
            tc.tile_pool(name="utmp", bufs=2) as upool,
            tc.tile_pool(name="obuf", bufs=2) as opool,
            tc.tile_pool(name="psacc", bufs=1, space="PSUM") as psacc,
            tc.tile_pool(name="pssc", bufs=1, space="PSUM") as pssc,
        ):
            # PE warm-up: HAM keeps the PE clock-gated at 1.2 GHz until it
            # sees ~3.4us of sustained activity, so burn the initial DMA
            # wait on dummy matmuls over a zeroed tile (result never read;
            # the psum tag is recycled by the first real accumulator).
            zt = const.tile([P, 640], bf16, name="zt", tag="zt")
            nc.gpsimd.memset(zt, 0)
            warm = psacc.tile([P, S], f32, name="warm", tag="acc0")
            for _ in range(8):
                nc.tensor.matmul(warm, zt[:, 0:P], zt[:, P:P + S],
                                 start=True, stop=True)

            # Factor loads ride the fast HWDGE rings, one packed DMA per
            # matrix (DMA issue costs ~650ns of engine time and completion
            # ~2us of latency, so fewer+bigger wins at startup): gate
            # factors lead the scalar ring in parallel with the first
            # weight quad on sync; up/down factors interleave with the
            # weight stream later (needed ~30us in).
            fac = {}

            def load_fac(nm, dram, eng):
                t = const.tile(list(dram.shape), bf16, name=nm, tag=nm)
                if eng is not None:
                    eng.dma_start(t, dram[:])
                fac[nm] = t

            load_fac("gF", gFac, nc.scalar)
            load_fac("uF", uFac, None)
            load_fac("dF", dFac, None)

            # x in four independent tiles so each main matmul depends only
            # on its own quarter's DMA (a single tile would make the first
            # matmuls wait for the LAST x transfer). First quarter next on
            # the scalar ring; the rest interleave with the weight stream.
            x_t = [const.tile([P, 4, S], bf16, name=f"x{i}", tag=f"x{i}")
                   for i in range(4)]
            nc.scalar.dma_start(x_t[0], x[:, 0:4])

            def xs(kc):
                return x_t[kc // 4][:, kc % 4]

            h_sb = const.tile([P, KF, S], bf16, name="h", tag="h")

            # pass list: (kind, fgroup, n_chunks, weight dram, factors)
            passes = []
            for fg in range(F // FG):
                passes.append(("g", fg, KD, gTp, "gF"))
            for fg in range(F // FG):
                passes.append(("u", fg, KD, uTp, "uF"))
            for mg in range(D // FG):
                passes.append(("d", mg, KF, dTp, "dF"))
            # job = (pass, first chunk, n chunks). Pass 0 starts [2, 2] so
            # the startup chain (factors -> sc -> dequant -> mains) clears
            # on a 256 KiB half-quad instead of a full 512 KiB quad, and
            # the second half-dequant hides under the first mains.
            jobs = []
            for pi, ps in enumerate(passes):
                sizes = [2, 2] + [4] * (ps[2] // 4 - 1) if pi == 0 \
                    else [4] * (ps[2] // 4)
                c0 = 0
                for nch in sizes:
                    jobs.append((pi, c0, nch))
                    c0 += nch
            NJ = len(jobs)

            wt_tiles = {}

            def emit_wt(J):
                pi, c0, nch = jobs[J]
                kind, fg, nk, wdram, _ = passes[pi]
                # always a full-quad allocation (uniform pool slot shape)
                # even when the job covers fewer chunks
                wt = wpool.tile([P, 4, FG], bf16, name="wt", tag="wt")
                # both halves of the split first quad go on sync so the
                # scalar ring is free to deliver gFac + all of x early.
                weng = nc.sync if (J < 2 or J % 2 == 0) else nc.scalar
                qbase = fg * (nk // 4) + c0 // 4
                weng.dma_start(wt[:, 0:nch],
                               wdram[qbase, :, c0 % 4:c0 % 4 + nch])
                wt_tiles[J] = wt
                # the remaining x quarters follow gFac+x0 down the scalar
                # ring ahead of everything else there; up/down factors
                # interleave later (needed ~40us/~70us in).
                if J in (1, 2, 3):
                    nc.scalar.dma_start(x_t[J], x[:, 4 * J:4 * J + 4])
                elif J == 8:
                    nc.scalar.dma_start(fac["uF"], uFac[:])
                elif J == 10:
                    nc.scalar.dma_start(fac["dF"], dFac[:])

            sc_tiles = {}

            def emit_sc(J):
                pi, c0, nch = jobs[J]
                kind, fg, nk, _, Fn = passes[pi]
                sc4 = pssc.tile([P, 4, FG], f32, name="sc", tag="sc")
                Ft = fac[Fn]
                aoff = (nk // 4) * P + fg * FG  # A^T block after B strips
                for i in range(nch):
                    c = c0 + i
                    s, g = c % 4, c // 4
                    nc.tensor.matmul(
                        sc4[:, i],
                        Ft[32 * s:32 * (s + 1), g * P:(g + 1) * P],
                        Ft[32 * s:32 * (s + 1), aoff:aoff + FG],
                        start=True, stop=True,
                        tile_position=(32 * s, 0),
                    )
                sc_tiles[J] = sc4

            wr_tiles = {}

            def emit_dq(J):
                nch = jobs[J][2]
                wr = rpool.tile([P, 4, FG], bf16, name="wr", tag="wr")
                nc.vector.tensor_mul(out=wr[:, 0:nch],
                                     in0=wt_tiles.pop(J)[:, 0:nch],
                                     in1=sc_tiles.pop(J)[:, 0:nch])
                wr_tiles[J] = wr

            def finish(pi, acc):
                kind, fg = passes[pi][0], passes[pi][1]
                if kind == "g":
                    for fi in range(4):
                        nc.scalar.activation(h_sb[:, fg * 4 + fi], acc[fi],
                                             silu)
                elif kind == "u":
                    # psum -> bf16 SBUF on ACT, then a pure-SBUF packed
                    # DVE multiply: keeps DVE free for the dequants.
                    ut = upool.tile([P, 4, S], bf16, name="ut", tag="ut")
                    for fi in range(4):
                        f = fg * 4 + fi
                        nc.scalar.copy(ut[:, fi], acc[fi])
                        nc.vector.tensor_mul(out=h_sb[:, f], in0=h_sb[:, f],
                                             in1=ut[:, fi])
                else:
                    ot = opool.tile([P, 4, S], bf16, name="ot", tag="ot")
                    for fi in range(4):
                        nc.scalar.copy(ot[:, fi], acc[fi])
                    weng = nc.sync if fg % 2 == 0 else nc.scalar
                    weng.dma_start(out[fg], ot)

            DMA_AHEAD = 5
            LOOK = 1
            for J in range(DMA_AHEAD):
                emit_wt(J)
            for J in range(LOOK):
                emit_sc(J)
                emit_dq(J)

            acc = None
            for J, (pi, c0, nch) in enumerate(jobs):
                kind, fg, nk = passes[pi][0], passes[pi][1], passes[pi][2]
                if c0 == 0:
                    acc = [psacc.tile([P, S], f32, name=f"acc{i}",
                                      tag=f"acc{i}") for i in range(4)]
                if J + DMA_AHEAD < NJ:
                    emit_wt(J + DMA_AHEAD)
                if J + LOOK < NJ:
                    emit_sc(J + LOOK)
                    emit_dq(J + LOOK)
                wr = wr_tiles.pop(J)
                if J == NJ - 1:
                    # kernel tail: fi-major so each output bank finishes 4
                    # matmuls apart — copies and stores overlap the last
                    # matmuls instead of serializing after them. Copies
                    # split ACT/DVE, stores split across both HWDGE rings.
                    for fi in range(4):
                        for c in range(nch):
                            nc.tensor.matmul(
                                acc[fi],
                                wr[:, c, fi * P:(fi + 1) * P],
                                h_sb[:, c0 + c],
                                start=False,
                                stop=(c == nch - 1),
                            )
                        ot = opool.tile([P, S], bf16, name="otl",
                                        tag=f"otl{fi}")
                        if fi % 2 == 0:
                            nc.scalar.copy(ot, acc[fi])
                        else:
                            nc.vector.tensor_copy(out=ot, in_=acc[fi])
                        weng = nc.sync if fi % 2 == 0 else nc.scalar
                        weng.dma_start(out[fg, :, fi], ot)
                    continue
                for c in range(nch):
                    kc = c0 + c
                    rhs = xs(kc) if kind in "gu" else h_sb[:, kc]
                    for fi in range(4):
                        nc.tensor.matmul(
                            acc[fi],
                            wr[:, c, fi * P:(fi + 1) * P],
                            rhs,
                            start=(kc == 0 and c == 0),
                            stop=(c0 + nch == nk and c == nch - 1),
                        )
                if c0 + nch == nk:
                    finish(pi, acc)
    nc.finalize()
    return nc


def _prep_inputs(x, gate_snapped, gate_scale_A, gate_scale_B,
                 up_snapped, up_scale_A, up_scale_B,
                 down_snapped, down_scale_A, down_scale_B):
    bf = ml_dtypes.bfloat16
    x2 = np.ascontiguousarray(
        np.asarray(x, dtype=np.float32).reshape(D, S).astype(bf)
        .reshape(KD, P, S).transpose(1, 0, 2))

    def quad_tile(wT_bf, npass):
        # wT [K, W] bf16 (contraction-major) -> [npass*quads, 128, 4, 512]
        K, W = wT_bf.shape
        nq = K // (4 * P)
        t = wT_bf.reshape(nq, 4, P, npass, FG).transpose(3, 0, 2, 1, 4)
        return np.ascontiguousarray(t.reshape(npass * nq, P, 4, FG))

    def pack_fac(Bmat, Amat, nk):
        # one packed [128, nk*32 + w] factor block per matrix:
        # cols [0, nk*32): B strips (strip i = rows 32i.., chunk 4q+i at
        # col-group q); cols [nk*32, ..): A^T replicated on all 4 strips.
        b = np.asarray(Bmat, dtype=np.float32).reshape(R, nk // 4, 4, P)
        at = np.asarray(Amat, dtype=np.float32).T.astype(bf)  # [R, w]
        w = at.shape[1]
        o = np.empty((4 * R, nk // 4 * P + w), dtype=bf)
        for i in range(4):
            o[R * i:R * (i + 1), :nk // 4 * P] = \
                b[:, :, i, :].astype(bf).reshape(R, nk // 4 * P)
            o[R * i:R * (i + 1), nk // 4 * P:] = at
        return o

    gs = np.asarray(gate_snapped, dtype=np.float32)
    us = np.asarray(up_snapped, dtype=np.float32)
    ds = np.asarray(down_snapped, dtype=np.float32)
    gB_f = np.asarray(gate_scale_B, dtype=np.float32)
    uB_f = np.asarray(up_scale_B, dtype=np.float32)
    dB_f = np.asarray(down_scale_B, dtype=np.float32)
    gA_f = np.asarray(gate_scale_A, dtype=np.float32)
    uA_f = np.asarray(up_scale_A, dtype=np.float32)
    dA_f = np.asarray(down_scale_A, dtype=np.float32)

    in_maps = []
    for c in range(NCORES):
        lo, hi = c * F, (c + 1) * F
        in_maps.append({
            "x": x2,
            "gTp": quad_tile(gs[lo:hi].T.astype(bf), F // FG),
            "uTp": quad_tile(us[lo:hi].T.astype(bf), F // FG),
            "dTp": quad_tile(ds[:, lo:hi].T.astype(bf), D // FG),
            "gFac": pack_fac(gB_f, gA_f[lo:hi], KD),
            "uFac": pack_fac(uB_f, uA_f[lo:hi], KD),
            "dFac": pack_fac(dB_f[:, lo:hi], dA_f, KF),
        })
    return in_maps


def run(trace=False, **inputs):
    if "nc" not in _CACHE:
        _CACHE["nc"] = _build()
    nc = _CACHE["nc"]
    in_maps = _prep_inputs(**inputs)
    try:
        res = run_bass_kernel_spmd(nc, in_maps, list(range(NCORES)), trace=trace)
    except Exception:
        # A transient device flake (NRT_EXEC_UNIT_UNRECOVERABLE) poisons the
        # PJRT client for the process; tearing the backend down and
        # reconnecting recovers it the same way a fresh process does.
        try:
            import jax.extend.backend
            jax.extend.backend.clear_backends()
        except Exception:
            pass
        res = run_bass_kernel_spmd(nc, in_maps, list(range(NCORES)), trace=trace)
    partial = np.zeros((4, P, 4, S), dtype=np.float32)
    for c in range(NCORES):
        partial += res.results[c]["out"].astype(np.float32)
    full = partial.transpose(0, 2, 1, 3).reshape(D, S)
    return full.reshape(1, D, 1, S), res


def kernel(**inputs):
    out, _ = run(trace=False, **inputs)
    return out


if __name__ == "__main__":
    rng = np.random.default_rng(0)
    ins = {
        "x": rng.standard_normal((1, D, 1, S)).astype(np.float32),
        "gate_snapped": (rng.standard_normal((FF, D)) * 0.02).astype(np.float32),
        "gate_scale_A": (rng.standard_normal((FF, R)) * 0.1).astype(np.float32),
        "gate_scale_B": (rng.standard_normal((R, D)) * 0.1).astype(np.float32),
        "up_snapped": (rng.standard_normal((FF, D)) * 0.02).astype(np.float32),
        "up_scale_A": (rng.standard_normal((FF, R)) * 0.1).astype(np.float32),
        "up_scale_B": (rng.standard_normal((R, D)) * 0.1).astype(np.float32),
        "down_snapped": (rng.standard_normal((D, FF)) * 0.02).astype(np.float32),
        "down_scale_A": (rng.standard_normal((D, R)) * 0.1).astype(np.float32),
        "down_scale_B": (rng.standard_normal((R, FF)) * 0.1).astype(np.float32),
    }
    out = kernel(**ins)
    print("kernel ran, out shape", out.shape, "mean abs", np.abs(out).mean())


# revision 33
# speedup vs baseline: 1.1910x; 1.0233x over previous
"""Trainium2 Bass kernel for FFNWithScales (SwiGLU MLP with low-rank dequant scales).

Reference computation (all fp32):
    gate_eff = gate_snapped * (gate_scale_A @ gate_scale_B)       # [8192, 2048]
    up_eff   = up_snapped   * (up_scale_A   @ up_scale_B)         # [8192, 2048]
    down_eff = down_snapped * (down_scale_A @ down_scale_B)       # [2048, 8192]
    h   = silu(gate_eff @ x) * (up_eff @ x)                       # [8192, 512]
    out = down_eff @ h                                            # [2048, 512]

Sharding (8 cores, tensor-parallel on d_ff): core c owns d_ff rows
[c*1024, (c+1)*1024) of gate/up (and the matching columns of down).
Each core computes a full-[2048, 512] partial of the down projection;
bf16 partials are summed in fp32 on the host (the all-reduce step).

Device notes:
  - PE matmul computes psum[M,N] = lhsT[K,M].T @ rhs[K,N] with K on
    partitions. The host ships everything bf16 in final device layout:
    snapped weights are pre-transposed AND pre-tiled so each weight DMA
    is one fully contiguous [128, 4, 512] (512 KiB) "quad" = 4 K-chunks
    of one 512-wide output group. bf16 weights halve the dominant HBM
    stream vs fp32 (measured end-to-end error ~7e-3 of output absmax,
    threshold 2e-2).
  - Work unit = quad job: one 512 KiB weight DMA, one 4-way row-packed
    rank-32 scale matmul (strips at tile_position (32i,0) run
    concurrently in the PE array, output [128,4,512] psum = 4 banks),
    one full-tile DVE dequant multiply (bf16), then 16 [128,128]x
    [128,512] bf16 main matmuls accumulating into 4 psum banks.
    PSUM budget: 4 acc banks + 4 scale banks = all 8.
  - DMA rings: sync+scalar HWDGE alternate the weight stream (and carry
    x / output stores interleaved); the six small scale-factor loads go
    on the gpsimd SWDGE ring so the startup critical path is just
    factors(gpsimd) -> scale mm -> dequant in parallel with wt+x DMAs.
  - Up-projection finish copies psum to bf16 SBUF on ACT, then the
    SwiGLU multiply runs SBUF-only on DVE in 2x packed mode, keeping
    DVE's steady-state load (the dequants) off the critical path at
    pass boundaries.
"""

import numpy as np
import ml_dtypes

import concourse.bass as bass
from concourse import bacc
import concourse.mybir as mybir
from concourse.tile import TileContext
from concourse.bass_utils import run_bass_kernel_spmd

P = 128
D = 2048        # d_model
FF = 8192       # d_ff (global)
S = 512         # sequence
R = 32          # rank
NCORES = 8
F = FF // NCORES          # 1024 local d_ff rows
KD = D // P               # 16 d_model chunks
KF = F // P               # 8 local d_ff chunks
FG = 512                  # free-dim group (psum bank width)

f32 = mybir.dt.float32
bf16 = mybir.dt.bfloat16

_CACHE = {}


def _build():
    nc = bacc.Bacc()
    x = nc.declare_dram_parameter("x", [P, KD, S], bf16, isOutput=False)
    # quad-tiled weights: [pass*quads, 128, 4 k-chunks, 512 out-cols]
    gTp = nc.declare_dram_parameter("gTp", [8, P, 4, FG], bf16, isOutput=False)
    uTp = nc.declare_dram_parameter("uTp", [8, P, 4, FG], bf16, isOutput=False)
    dTp = nc.declare_dram_parameter("dTp", [8, P, 4, FG], bf16, isOutput=False)
    # Packed per-matrix scale factors, one DMA each: cols [0, nk*32) hold
    # B strips (strip i = partitions 32i..32i+31, chunk 4q+i at col q*128),
    # the rest holds A^T replicated on all 4 strips.
    gFac = nc.declare_dram_parameter("gFac", [P, KD * R + F], bf16, isOutput=False)
    uFac = nc.declare_dram_parameter("uFac", [P, KD * R + F], bf16, isOutput=False)
    dFac = nc.declare_dram_parameter("dFac", [P, KF * R + D], bf16, isOutput=False)
    out = nc.declare_dram_parameter("out", [4, P, 4, S], bf16, isOutput=True)

    silu = mybir.ActivationFunctionType.Silu

    with TileContext(nc) as tc:
        with (
            tc.tile_pool(name="const", bufs=1) as const,
            tc.tile_pool(name="wstream", bufs=6) as wpool,
            tc.tile_pool(name="wready", bufs=3) as rpool,
            tc.tile_pool(name="utmp", bufs=2) as upool,
            tc.tile_pool(name="obuf", bufs=2) as opool,
            tc.tile_pool(name="psacc", bufs=1, space="PSUM") as psacc,
            tc.tile_pool(name="pssc", bufs=1, space="PSUM") as pssc,
        ):
            # PE warm-up: HAM keeps the PE clock-gated at 1.2 GHz until it
            # sees a full ~3.4us window of sustained activity, so burn the
            # initial DMA wait on dummy matmuls over a zeroed tile (result
            # never read; the psum tag is recycled by the first real
            # accumulator). 9 dummies > one full window at the cold rate;
            # more fill the later startup-chain bubbles (emitted below).
            zt = const.tile([P, 640], bf16, name="zt", tag="zt")
            nc.gpsimd.memset(zt, 0)
            # ACT warm-up: the silu table load (~1.3us) otherwise happens
            # lazily right at the first gate-pass finish, on the critical
            # path of the next pass's accumulator release.
            at = const.tile([P, 2], bf16, name="at", tag="at")
            nc.scalar.copy(at[:, 0:1], zt[:, 0:1])
            nc.scalar.activation(at[:, 1:2], zt[:, 0:1],
                                 mybir.ActivationFunctionType.Silu)

            warm = psacc.tile([P, S], f32, name="warm", tag="acc0")

            def emit_warm(n):
                for _ in range(n):
                    nc.tensor.matmul(warm, zt[:, 0:P], zt[:, P:P + S],
                                     start=True, stop=True)

            emit_warm(9)

            # Factor loads ride the fast HWDGE rings, one packed DMA per
            # matrix (DMA issue costs ~650ns of engine time and completion
            # ~2us of latency, so fewer+bigger wins at startup): gate
            # factors lead the scalar ring in parallel with the first
            # weight quad on sync; up/down factors interleave with the
            # weight stream later (needed ~30us in).
            fac = {}

            def load_fac(nm, dram, eng):
                t = const.tile(list(dram.shape), bf16, name=nm, tag=nm)
                if eng is not None:
                    eng.dma_start(t, dram[:])
                fac[nm] = t

            load_fac("gF", gFac, nc.scalar)
            load_fac("uF", uFac, None)
            load_fac("dF", dFac, None)

            # x in four independent tiles so each main matmul depends only
            # on its own quarter's DMA (a single tile would make the first
            # matmuls wait for the LAST x transfer). First quarter next on
            # the scalar ring; the rest interleave with the weight stream.
            x_t = [const.tile([P, 4, S], bf16, name=f"x{i}", tag=f"x{i}")
                   for i in range(4)]
            nc.scalar.dma_start(x_t[0], x[:, 0:4])

            def xs(kc):
                return x_t[kc // 4][:, kc % 4]

            h_sb = const.tile([P, KF, S], bf16, name="h", tag="h")

            # pass list: (kind, fgroup, n_chunks, weight dram, factors)
            passes = []
            for fg in range(F // FG):
                passes.append(("g", fg, KD, gTp, "gF"))
            for fg in range(F // FG):
                passes.append(("u", fg, KD, uTp, "uF"))
            for mg in range(D // FG):
                passes.append(("d", mg, KF, dTp, "dF"))
            # job = (pass, first chunk, n chunks). Pass 0 starts [2, 2] so
            # the startup chain (factors -> sc -> dequant -> mains) clears
            # on a 256 KiB half-quad instead of a full 512 KiB quad, and
            # the second half-dequant hides under the first mains.
            jobs = []
            for pi, ps in enumerate(passes):
                sizes = [2, 2] + [4] * (ps[2] // 4 - 1) if pi == 0 \
                    else [4] * (ps[2] // 4)
                c0 = 0
                for nch in sizes:
                    jobs.append((pi, c0, nch))
                    c0 += nch
            NJ = len(jobs)

            wt_tiles = {}

            def emit_wt(J):
                pi, c0, nch = jobs[J]
                kind, fg, nk, wdram, _ = passes[pi]
                # always a full-quad allocation (uniform pool slot shape)
                # even when the job covers fewer chunks
                wt = wpool.tile([P, 4, FG], bf16, name="wt", tag="wt")
                # ALL weight transfers ride the sync ring: a weight-DMA
                # issue parked on the ACT engine head-of-line blocks the
                # pass-finish silu/copies behind it (HWDGE is FIFO per
                # issuing engine). Scalar carries only gFac/x/factors
                # (early) and the output stores (late).
                qbase = fg * (nk // 4) + c0 // 4
                nc.sync.dma_start(wt[:, 0:nch],
                                  wdram[qbase, :, c0 % 4:c0 % 4 + nch])
                wt_tiles[J] = wt
                # the remaining x quarters follow gFac+x0 down the scalar
                # ring ahead of everything else there; up/down factors
                # interleave later (needed ~40us/~70us in).
                if J in (1, 2, 3):
                    nc.scalar.dma_start(x_t[J], x[:, 4 * J:4 * J + 4])
                elif J == 8:
                    nc.scalar.dma_start(fac["uF"], uFac[:])
                elif J == 10:
                    nc.scalar.dma_start(fac["dF"], dFac[:])

            sc_tiles = {}

            def emit_sc(J):
                pi, c0, nch = jobs[J]
                kind, fg, nk, _, Fn = passes[pi]
                sc4 = pssc.tile([P, 4, FG], f32, name="sc", tag="sc")
                Ft = fac[Fn]
                aoff = (nk // 4) * P + fg * FG  # A^T block after B strips
                for i in range(nch):
                    c = c0 + i
                    s, g = c % 4, c // 4
                    nc.tensor.matmul(
                        sc4[:, i],
                        Ft[32 * s:32 * (s + 1), g * P:(g + 1) * P],
                        Ft[32 * s:32 * (s + 1), aoff:aoff + FG],
                        start=True, stop=True,
                        tile_position=(32 * s, 0),
                    )
                sc_tiles[J] = sc4

            wr_tiles = {}

            def emit_dq(J):
                nch = jobs[J][2]
                wr = rpool.tile([P, 4, FG], bf16, name="wr", tag="wr")
                wt, sc4 = wt_tiles.pop(J), sc_tiles.pop(J)
                # two halves so the first half's completion semaphore
                # (DVE->PE visibility costs ~2us) fires a half-dequant
                # earlier than the main matmuls that consume it need it —
                # a single full dequant leaves the whole stream
                # semaphore-cadence-bound (~4.1us chain vs 3.46us window).
                for h0, hn in ([(0, nch)] if nch <= 2 else [(0, 2), (2, 2)]):
                    nc.vector.tensor_mul(out=wr[:, h0:h0 + hn],
                                         in0=wt[:, h0:h0 + hn],
                                         in1=sc4[:, h0:h0 + hn])
                wr_tiles[J] = wr

            def finish(pi, acc):
                kind, fg = passes[pi][0], passes[pi][1]
                if kind == "g":
                    for fi in range(4):
                        nc.scalar.activation(h_sb[:, fg * 4 + fi], acc[fi],
                                             silu)
                elif kind == "u":
                    # psum -> bf16 SBUF on ACT, then a pure-SBUF packed
                    # DVE multiply: keeps DVE free for the dequants.
                    ut = upool.tile([P, 4, S], bf16, name="ut", tag="ut")
                    for fi in range(4):
                        f = fg * 4 + fi
                        nc.scalar.copy(ut[:, fi], acc[fi])
                        nc.vector.tensor_mul(out=h_sb[:, f], in0=h_sb[:, f],
                                             in1=ut[:, fi])
                else:
                    # stores ride scalar: parking one on sync would
                    # head-of-line block the weight stream there.
                    ot = opool.tile([P, 4, S], bf16, name="ot", tag="ot")
                    for fi in range(4):
                        nc.scalar.copy(ot[:, fi], acc[fi])
                    nc.scalar.dma_start(out[fg], ot)

            DMA_AHEAD = 5
            LOOK = 1
            for J in range(DMA_AHEAD):
                emit_wt(J)
            for J in range(LOOK):
                emit_sc(J)
                emit_dq(J)
            # keep the PE busy across the startup chain's two serial
            # dependencies (first dequant, then its completion semaphore)
            # so HAM stays unthrottled into the real stream
            emit_warm(3)

            acc = None
            for J, (pi, c0, nch) in enumerate(jobs):
                kind, fg, nk = passes[pi][0], passes[pi][1], passes[pi][2]
                if c0 == 0:
                    acc = [psacc.tile([P, S], f32, name=f"acc{i}",
                                      tag=f"acc{i}") for i in range(4)]
                if J + DMA_AHEAD < NJ:
                    emit_wt(J + DMA_AHEAD)
                if J + LOOK < NJ:
                    emit_sc(J + LOOK)
                    emit_dq(J + LOOK)
                if J == 0:
                    emit_warm(3)
                wr = wr_tiles.pop(J)
                if J == NJ - 1:
                    # kernel tail: fi-major so each output bank finishes 4
                    # matmuls apart — copies and stores overlap the last
                    # matmuls instead of serializing after them. Copies
                    # split ACT/DVE, stores split across both HWDGE rings.
                    for fi in range(4):
                        for c in range(nch):
                            nc.tensor.matmul(
                                acc[fi],
                                wr[:, c, fi * P:(fi + 1) * P],
                                h_sb[:, c0 + c],
                                start=False,
                                stop=(c == nch - 1),
                            )
                        ot = opool.tile([P, S], bf16, name="otl",
                                        tag=f"otl{fi}")
                        if fi % 2 == 0:
                            nc.scalar.copy(ot, acc[fi])
                        else:
                            nc.vector.tensor_copy(out=ot, in_=acc[fi])
                        weng = nc.sync if fi % 2 == 0 else nc.scalar
                        weng.dma_start(out[fg, :, fi], ot)
                    continue
                for c in range(nch):
                    kc = c0 + c
                    rhs = xs(kc) if kind in "gu" else h_sb[:, kc]
                    for fi in range(4):
                        nc.tensor.matmul(
                            acc[fi],
                            wr[:, c, fi * P:(fi + 1) * P],
                            rhs,
                            start=(kc == 0 and c == 0),
                            stop=(c0 + nch == nk and c == nch - 1),
                        )
                if c0 + nch == nk:
                    finish(pi, acc)
    nc.finalize()
    return nc


def _prep_inputs(x, gate_snapped, gate_scale_A, gate_scale_B,
                 up_snapped, up_scale_A, up_scale_B,
                 down_snapped, down_scale_A, down_scale_B):
    bf = ml_dtypes.bfloat16
    x2 = np.ascontiguousarray(
        np.asarray(x, dtype=np.float32).reshape(D, S).astype(bf)
        .reshape(KD, P, S).transpose(1, 0, 2))

    def quad_tile(wT_bf, npass):
        # wT [K, W] bf16 (contraction-major) -> [npass*quads, 128, 4, 512]
        K, W = wT_bf.shape
        nq = K // (4 * P)
        t = wT_bf.reshape(nq, 4, P, npass, FG).transpose(3, 0, 2, 1, 4)
        return np.ascontiguousarray(t.reshape(npass * nq, P, 4, FG))

    def pack_fac(Bmat, Amat, nk):
        # one packed [128, nk*32 + w] factor block per matrix:
        # cols [0, nk*32): B strips (strip i = rows 32i.., chunk 4q+i at
        # col-group q); cols [nk*32, ..): A^T replicated on all 4 strips.
        b = np.asarray(Bmat, dtype=np.float32).reshape(R, nk // 4, 4, P)
        at = np.asarray(Amat, dtype=np.float32).T.astype(bf)  # [R, w]
        w = at.shape[1]
        o = np.empty((4 * R, nk // 4 * P + w), dtype=bf)
        for i in range(4):
            o[R * i:R * (i + 1), :nk // 4 * P] = \
                b[:, :, i, :].astype(bf).reshape(R, nk // 4 * P)
            o[R * i:R * (i + 1), nk // 4 * P:] = at
        return o

    gs = np.asarray(gate_snapped, dtype=np.float32)
    us = np.asarray(up_snapped, dtype=np.float32)
    ds = np.asarray(down_snapped, dtype=np.float32)
    gB_f = np.asarray(gate_scale_B, dtype=np.float32)
    uB_f = np.asarray(up_scale_B, dtype=np.float32)
    dB_f = np.asarray(down_scale_B, dtype=np.float32)
    gA_f = np.asarray(gate_scale_A, dtype=np.float32)
    uA_f = np.asarray(up_scale_A, dtype=np.float32)
    dA_f = np.asarray(down_scale_A, dtype=np.float32)

    in_maps = []
    for c in range(NCORES):
        lo, hi = c * F, (c + 1) * F
        in_maps.append({
            "x": x2,
            "gTp": quad_tile(gs[lo:hi].T.astype(bf), F // FG),
            "uTp": quad_tile(us[lo:hi].T.astype(bf), F // FG),
            "dTp": quad_tile(ds[:, lo:hi].T.astype(bf), D // FG),
            "gFac": pack_fac(gB_f, gA_f[lo:hi], KD),
            "uFac": pack_fac(uB_f, uA_f[lo:hi], KD),
            "dFac": pack_fac(dB_f[:, lo:hi], dA_f, KF),
        })
    return in_maps


def run(trace=False, **inputs):
    if "nc" not in _CACHE:
        _CACHE["nc"] = _build()
    nc = _CACHE["nc"]
    in_maps = _prep_inputs(**inputs)
    try:
        res = run_bass_kernel_spmd(nc, in_maps, list(range(NCORES)), trace=trace)
    except Exception:
        # A transient device flake (NRT_EXEC_UNIT_UNRECOVERABLE) poisons the
        # PJRT client for the process; tearing the backend down and
        # reconnecting recovers it the same way a fresh process does.
        try:
            import jax.extend.backend
            jax.extend.backend.clear_backends()
        except Exception:
            pass
        res = run_bass_kernel_spmd(nc, in_maps, list(range(NCORES)), trace=trace)
    partial = np.zeros((4, P, 4, S), dtype=np.float32)
    for c in range(NCORES):
        partial += res.results[c]["out"].astype(np.float32)
    full = partial.transpose(0, 2, 1, 3).reshape(D, S)
    return full.reshape(1, D, 1, S), res


def kernel(**inputs):
    out, _ = run(trace=False, **inputs)
    return out


if __name__ == "__main__":
    rng = np.random.default_rng(0)
    ins = {
        "x": rng.standard_normal((1, D, 1, S)).astype(np.float32),
        "gate_snapped": (rng.standard_normal((FF, D)) * 0.02).astype(np.float32),
        "gate_scale_A": (rng.standard_normal((FF, R)) * 0.1).astype(np.float32),
        "gate_scale_B": (rng.standard_normal((R, D)) * 0.1).astype(np.float32),
        "up_snapped": (rng.standard_normal((FF, D)) * 0.02).astype(np.float32),
        "up_scale_A": (rng.standard_normal((FF, R)) * 0.1).astype(np.float32),
        "up_scale_B": (rng.standard_normal((R, D)) * 0.1).astype(np.float32),
        "down_snapped": (rng.standard_normal((D, FF)) * 0.02).astype(np.float32),
        "down_scale_A": (rng.standard_normal((D, R)) * 0.1).astype(np.float32),
        "down_scale_B": (rng.standard_normal((R, FF)) * 0.1).astype(np.float32),
    }
    out = kernel(**ins)
    print("kernel ran, out shape", out.shape, "mean abs", np.abs(out).mean())
